# revision 11
# baseline (speedup 1.0000x reference)
"""Trainium2 Bass kernel for the autoregressive policy head (nn_ADM_6511170421537).

Structure (per core, pure data parallelism over 8 cores):
  trunk:  h = relu(x@sW0+b) -> relu(@sW1+b) -> relu(@sW2+b)          [B,256]
  steps i=0..7 (sequential in i, batch-parallel):
      x1 = relu(h@W_in[i][:256] + means[:i]@W_in[i][256:256+i] + b_in[i])
      x2 = relu(x1@W_h[i] + b_h[i])
      (mean_i, ls_i) = relu(x2@W_out[i] + b_out[i])
  epilogue (batched over the 8 steps, fp32):
      log_std = min(ls, 2);  std = exp(log_std)
      sample  = mean + std*eps
      logp    = -0.5*eps^2 - log_std - 0.5*log(2pi)   (== reference algebra)

Layout: everything feature-major on chip ([features->partitions, batch->free]).
The host transposes inputs/eps on the way in and the three outputs on the way
out so every DMA moves long contiguous lines.  Matmuls run in bf16 (weights
and activations bf16, PSUM accumulation fp32); the epilogue math is fp32.
"""

import os

os.environ.setdefault("MYCRO_LOCAL_CACHE", "1")

import numpy as np
from contextlib import ExitStack

import concourse.bass as bass
import concourse.bacc as bacc
import concourse.mybir as mybir
import concourse.tile as tile
from concourse.bass_utils import run_bass_kernel_spmd

# ---- problem constants (hardcoded; kernel.py must be self-contained) ----
B = 65536
IN_DIM = 64
HID = 256
D = 8
NCORES = 8
BC = B // NCORES          # 8192 rows per core
BT = 512                  # batch tile (one fp32 PSUM bank of free dim)
NT = BC // BT             # 16 tiles per core
WAVE = 8                  # tiles per wave (bounds SBUF liveness)
LOG_2PI = float(np.log(2.0 * np.pi))

F32 = mybir.dt.float32
BF16 = mybir.dt.bfloat16
RELU = mybir.ActivationFunctionType.Relu
EXP = mybir.ActivationFunctionType.Exp
ADD = mybir.AluOpType.add
MAX = mybir.AluOpType.max
MIN = mybir.AluOpType.min
MULT = mybir.AluOpType.mult

TRACE = False           # test.py flips this to get the NTFF profile
_NC_CACHE = {}


def _build_bass():
    nc = bacc.Bacc()

    xT = nc.declare_dram_parameter("xT", [IN_DIM, BC], BF16, isOutput=False)
    epsT = nc.declare_dram_parameter("epsT", [D, BC], F32, isOutput=False)
    w0 = nc.declare_dram_parameter("w0", [IN_DIM, HID], BF16, isOutput=False)
    w1 = nc.declare_dram_parameter("w1", [128, 2 * HID], BF16, isOutput=False)
    w2 = nc.declare_dram_parameter("w2", [128, 2 * HID], BF16, isOutput=False)
    wi = nc.declare_dram_parameter("wi", [128, D * 2 * HID], BF16, isOutput=False)
    wx = nc.declare_dram_parameter("wx", [D - 1, D * HID], BF16, isOutput=False)
    wh = nc.declare_dram_parameter("wh", [128, D * 2 * HID], BF16, isOutput=False)
    wo = nc.declare_dram_parameter("wo", [128, D * 2 * 2], BF16, isOutput=False)
    b0 = nc.declare_dram_parameter("b0", [128, 2], F32, isOutput=False)
    b1 = nc.declare_dram_parameter("b1", [128, 2], F32, isOutput=False)
    b2 = nc.declare_dram_parameter("b2", [128, 2], F32, isOutput=False)
    bi = nc.declare_dram_parameter("bi", [128, D * 2], F32, isOutput=False)
    bh = nc.declare_dram_parameter("bh", [128, D * 2], F32, isOutput=False)
    bo = nc.declare_dram_parameter("bo", [2, D], F32, isOutput=False)
    omT = nc.declare_dram_parameter("omT", [D, BC], F32, isOutput=True)
    osT = nc.declare_dram_parameter("osT", [D, BC], F32, isOutput=True)
    olT = nc.declare_dram_parameter("olT", [D, BC], F32, isOutput=True)

    with tile.TileContext(nc) as tc, ExitStack() as ctx:
        wp = ctx.enter_context(tc.tile_pool(name="w", bufs=1))
        hpool = ctx.enter_context(tc.tile_pool(name="h", bufs=WAVE + 2))
        mlpool = ctx.enter_context(tc.tile_pool(name="ml", bufs=WAVE + 2))
        xpool = ctx.enter_context(tc.tile_pool(name="xin", bufs=3))
        tpool = ctx.enter_context(tc.tile_pool(name="tr", bufs=2))
        epool = ctx.enter_context(tc.tile_pool(name="ep", bufs=3))
        opool = ctx.enter_context(tc.tile_pool(name="out", bufs=2))
        ps2 = ctx.enter_context(tc.tile_pool(name="ps2", bufs=3, space="PSUM"))
        psh = ctx.enter_context(tc.tile_pool(name="psh", bufs=2, space="PSUM"))

        # ---- resident weights ----
        w0_s = wp.tile([IN_DIM, HID], BF16)
        nc.sync.dma_start(w0_s[:], w0[:])
        w1_s = wp.tile([128, 2, HID], BF16)
        nc.sync.dma_start(w1_s[:], w1[:].rearrange("p (k m) -> p k m", k=2))
        w2_s = wp.tile([128, 2, HID], BF16)
        nc.sync.dma_start(w2_s[:], w2[:].rearrange("p (k m) -> p k m", k=2))
        wi_s = wp.tile([128, D, 2, HID], BF16)
        nc.sync.dma_start(wi_s[:], wi[:].rearrange("p (i k m) -> p i k m", i=D, k=2))
        wx_s = wp.tile([D - 1, D, HID], BF16)
        nc.sync.dma_start(wx_s[:], wx[:].rearrange("j (i m) -> j i m", i=D))
        wh_s = wp.tile([128, D, 2, HID], BF16)
        nc.sync.dma_start(wh_s[:], wh[:].rearrange("p (i k m) -> p i k m", i=D, k=2))
        wo_s = wp.tile([128, D, 2, 2], BF16)
        nc.sync.dma_start(wo_s[:], wo[:].rearrange("p (i k c) -> p i k c", i=D, k=2))
        b0_s = wp.tile([128, 2], F32)
        nc.sync.dma_start(b0_s[:], b0[:])
        b1_s = wp.tile([128, 2], F32)
        nc.sync.dma_start(b1_s[:], b1[:])
        b2_s = wp.tile([128, 2], F32)
        nc.sync.dma_start(b2_s[:], b2[:])
        bi_s = wp.tile([128, D, 2], F32)
        nc.sync.dma_start(bi_s[:], bi[:].rearrange("p (i m) -> p i m", i=D))
        bh_s = wp.tile([128, D, 2], F32)
        nc.sync.dma_start(bh_s[:], bh[:].rearrange("p (i m) -> p i m", i=D))
        bo_s = wp.tile([2, D], F32)
        nc.sync.dma_start(bo_s[:], bo[:])

        def evac_act(dst, src, bias):
            nc.scalar.activation(dst, src, RELU, bias=bias)

        def evac_dve(dst, src, bias):
            nc.vector.tensor_scalar(dst, src, bias, 0.0, ADD, MAX)

        def trunk(t):
            xt = xpool.tile([IN_DIM, BT], BF16, tag="xt")
            nc.sync.dma_start(xt[:], xT[:, bass.ts(t, BT)])
            ps = ps2.tile([128, 2, BT], F32, tag="ps2")
            hp = tpool.tile([128, 2, BT], BF16, tag="hp")
            for mo in range(2):
                nc.tensor.matmul(
                    ps[:, mo, :], w0_s[:, bass.ts(mo, 128)], xt[:],
                    start=True, stop=True,
                )
            for mo in range(2):
                evac_act(hp[:, mo, :], ps[:, mo, :], b0_s[:, mo : mo + 1])
            ps_b = ps2.tile([128, 2, BT], F32, tag="ps2")
            hq = tpool.tile([128, 2, BT], BF16, tag="hq")
            for mo in range(2):
                nc.tensor.matmul(
                    ps_b[:, mo, :], w1_s[:, 0, bass.ts(mo, 128)], hp[:, 0, :],
                    start=True, stop=False,
                )
                nc.tensor.matmul(
                    ps_b[:, mo, :], w1_s[:, 1, bass.ts(mo, 128)], hp[:, 1, :],
                    start=False, stop=True,
                )
            for mo in range(2):
                evac_dve(hq[:, mo, :], ps_b[:, mo, :], b1_s[:, mo : mo + 1])
            ps_c = ps2.tile([128, 2, BT], F32, tag="ps2")
            h_t = hpool.tile([128, 2, BT], BF16, tag="h")
            for mo in range(2):
                nc.tensor.matmul(
                    ps_c[:, mo, :], w2_s[:, 0, bass.ts(mo, 128)], hq[:, 0, :],
                    start=True, stop=False,
                )
                nc.tensor.matmul(
                    ps_c[:, mo, :], w2_s[:, 1, bass.ts(mo, 128)], hq[:, 1, :],
                    start=False, stop=True,
                )
            for mo in range(2):
                evac_act(h_t[:, mo, :], ps_c[:, mo, :], b2_s[:, mo : mo + 1])
            return h_t

        def step(i, t, h_t, ml_t):
            ps = ps2.tile([128, 2, BT], F32, tag="ps2")
            x1 = tpool.tile([128, 2, BT], BF16, tag="x1")
            for mo in range(2):
                nc.tensor.matmul(
                    ps[:, mo, :], wi_s[:, i, 0, bass.ts(mo, 128)], h_t[:, 0, :],
                    start=True, stop=False,
                )
                nc.tensor.matmul(
                    ps[:, mo, :], wi_s[:, i, 1, bass.ts(mo, 128)], h_t[:, 1, :],
                    start=False, stop=(i == 0),
                )
                if i > 0:
                    nc.tensor.matmul(
                        ps[:, mo, :], wx_s[0:i, i, bass.ts(mo, 128)],
                        ml_t[0:i, :], start=False, stop=True,
                    )
            evac_act(x1[:, 0, :], ps[:, 0, :], bi_s[:, i, 0:1])
            evac_dve(x1[:, 1, :], ps[:, 1, :], bi_s[:, i, 1:2])

            ps_b = ps2.tile([128, 2, BT], F32, tag="ps2")
            x2 = tpool.tile([128, 2, BT], BF16, tag="x2")
            for mo in range(2):
                nc.tensor.matmul(
                    ps_b[:, mo, :], wh_s[:, i, 0, bass.ts(mo, 128)], x1[:, 0, :],
                    start=True, stop=False,
                )
                nc.tensor.matmul(
                    ps_b[:, mo, :], wh_s[:, i, 1, bass.ts(mo, 128)], x1[:, 1, :],
                    start=False, stop=True,
                )
            evac_act(x2[:, 0, :], ps_b[:, 0, :], bh_s[:, i, 0:1])
            evac_dve(x2[:, 1, :], ps_b[:, 1, :], bh_s[:, i, 1:2])

            pso = psh.tile([2, BT], F32, tag="psh")
            nc.tensor.matmul(
                pso[:], wo_s[:, i, 0, :], x2[:, 0, :], start=True, stop=False
            )
            nc.tensor.matmul(
                pso[:], wo_s[:, i, 1, :], x2[:, 1, :], start=False, stop=True
            )
            # head evac: both rows at base 0 (engine ops need 32-aligned
            # partition bases), then DMA-scatter into the ml rows.
            sm = xpool.tile([2, BT], BF16, tag="sm")
            evac_act(sm[:], pso[:], bo_s[:, i : i + 1])
            nc.sync.dma_start(ml_t[i : i + 1, :], sm[0:1, :])
            nc.sync.dma_start(ml_t[32 + i : 32 + i + 1, :], sm[1:2, :])

        def epilogue(t, ml_t):
            et = epool.tile([D, BT], F32, tag="et")
            nc.sync.dma_start(et[:], epsT[:, bass.ts(t, BT)])
            mean_f = opool.tile([D, BT], F32, tag="mean_f")
            nc.vector.tensor_copy(mean_f[:], ml_t[0:D, :])
            ls = opool.tile([D, BT], F32, tag="ls")
            nc.vector.tensor_single_scalar(ls[:], ml_t[32 : 32 + D, :], 2.0, MIN)
            st = opool.tile([D, BT], F32, tag="st")
            nc.scalar.activation(st[:], ls[:], EXP)
            smp = opool.tile([D, BT], F32, tag="smp")
            nc.vector.tensor_mul(smp[:], st[:], et[:])
            smp2 = opool.tile([D, BT], F32, tag="smp2")
            nc.vector.tensor_add(smp2[:], smp[:], mean_f[:])
            sq = opool.tile([D, BT], F32, tag="sq")
            nc.vector.tensor_mul(sq[:], et[:], et[:])
            lp = opool.tile([D, BT], F32, tag="lp")
            nc.vector.tensor_scalar(lp[:], sq[:], -0.5, -0.5 * LOG_2PI, MULT, ADD)
            lp2 = opool.tile([D, BT], F32, tag="lp2")
            nc.vector.tensor_sub(lp2[:], lp[:], ls[:])
            nc.sync.dma_start(omT[:, bass.ts(t, BT)], mean_f[:])
            nc.sync.dma_start(osT[:, bass.ts(t, BT)], smp2[:])
            nc.sync.dma_start(olT[:, bass.ts(t, BT)], lp2[:])

        for wv in range(NT // WAVE):
            tiles_ = list(range(wv * WAVE, (wv + 1) * WAVE))
            hs = {}
            mls = {}
            for t in tiles_:
                hs[t] = trunk(t)
                mls[t] = mlpool.tile([32 + D, BT], BF16, tag="ml", name=f"ml{t}")
            for i in range(D):
                for t in tiles_:
                    step(i, t, hs[t], mls[t])
                    if i == D - 1:
                        epilogue(t, mls[t])

    nc.compile()
    return nc


def _get_nc():
    if "nc" not in _NC_CACHE:
        _NC_CACHE["nc"] = _build_bass()
    return _NC_CACHE["nc"]


def kernel(**inputs):
    import ml_dtypes

    bf16 = ml_dtypes.bfloat16
    inp = {k: np.ascontiguousarray(np.asarray(v, dtype=np.float32)) for k, v in inputs.items()}
    x = inp["inputs"]
    eps = inp["eps"]
    W_in, b_in = inp["W_in"], inp["b_in"]
    W_h, b_h = inp["W_h"], inp["b_h"]
    W_out, b_out = inp["W_out"], inp["b_out"]

    def cb(a):
        return np.ascontiguousarray(a.astype(bf16))

    c = np.ascontiguousarray
    shared = {
        "w0": cb(inp["sW0"]),
        "w1": cb(inp["sW1"].reshape(2, 128, HID).transpose(1, 0, 2).reshape(128, -1)),
        "w2": cb(inp["sW2"].reshape(2, 128, HID).transpose(1, 0, 2).reshape(128, -1)),
        "wi": cb(W_in[:, :HID, :].reshape(D, 2, 128, HID).transpose(2, 0, 1, 3).reshape(128, -1)),
        "wx": cb(W_in[:, HID:, :].transpose(1, 0, 2).reshape(D - 1, -1)),
        "wh": cb(W_h.reshape(D, 2, 128, HID).transpose(2, 0, 1, 3).reshape(128, -1)),
        "wo": cb(W_out.reshape(D, 2, 128, 2).transpose(2, 0, 1, 3).reshape(128, -1)),
        "b0": c(inp["sb0"].reshape(2, 128).T),
        "b1": c(inp["sb1"].reshape(2, 128).T),
        "b2": c(inp["sb2"].reshape(2, 128).T),
        "bi": c(b_in.reshape(D, 2, 128).transpose(2, 0, 1).reshape(128, -1)),
        "bh": c(b_h.reshape(D, 2, 128).transpose(2, 0, 1).reshape(128, -1)),
        "bo": c(b_out.T),
    }

    in_maps = []
    for core in range(NCORES):
        sl = slice(core * BC, (core + 1) * BC)
        m = dict(shared)
        m["xT"] = cb(x[sl].T)
        m["epsT"] = c(eps[sl].T)
        in_maps.append(m)

    nc = _get_nc()
    kw = {}
    if TRACE:
        import shutil

        shutil.rmtree("/tmp/ktrace", ignore_errors=True)
        os.makedirs("/tmp/ktrace", exist_ok=True)
        kw = dict(trace=True, trace_cores=[0], tmpdir="/tmp/ktrace")
    res = run_bass_kernel_spmd(nc, in_maps, list(range(NCORES)), **kw)
    if TRACE:
        print(f"HW exec time: {res.exec_time_ns} ns")

    out_mean = np.concatenate([res.results[i]["omT"].T for i in range(NCORES)], axis=0)
    out_sample = np.concatenate([res.results[i]["osT"].T for i in range(NCORES)], axis=0)
    out_logp = np.concatenate([res.results[i]["olT"].T for i in range(NCORES)], axis=0)
    return out_mean, out_sample, out_logp


# revision 12
# speedup vs baseline: 1.2220x; 1.2220x over previous
"""Trainium2 Bass kernel for the autoregressive policy head (nn_ADM_6511170421537).

Structure (per core, pure data parallelism over 8 cores):
  trunk:  h = relu(x@sW0+b) -> relu(@sW1+b) -> relu(@sW2+b)          [B,256]
  steps i=0..7 (sequential in i, batch-parallel):
      x1 = relu(h@W_in[i][:256] + means[:i]@W_in[i][256:256+i] + b_in[i])
      x2 = relu(x1@W_h[i] + b_h[i])
      (mean_i, ls_i) = relu(x2@W_out[i] + b_out[i])
  epilogue (batched over the 8 steps, fp32):
      log_std = min(ls, 2);  std = exp(log_std)
      sample  = mean + std*eps
      logp    = -0.5*eps^2 - log_std - 0.5*log(2pi)   (== reference algebra)

Layout: feature-major on chip ([features->partitions, batch->free]); the host
transposes inputs/eps/outputs so every DMA moves contiguous lines.  Matmuls
run in bf16 (PSUM accumulates fp32), epilogue math in fp32.

Perf structure: batch tiles are processed in GROUPS of 4 (two PAIRS).  All
matmuls sharing a stationary operand are emitted back-to-back (weight-load
hides in the streaming of the previous matmul), a pair shares one 2-bank
PSUM tile so each PSUM->SBUF evacuation covers 2 tiles in one op (FD=1024),
and the tiny M=2 head matmuls of the 4 tiles in a group run CONCURRENTLY in
disjoint PE column groups via tile_position.
"""

import os

os.environ.setdefault("MYCRO_LOCAL_CACHE", "1")

import numpy as np
from contextlib import ExitStack

import concourse.bass as bass
import concourse.bacc as bacc
import concourse.mybir as mybir
import concourse.tile as tile
from concourse.bass_utils import run_bass_kernel_spmd

# ---- problem constants (hardcoded; kernel.py must be self-contained) ----
B = 65536
IN_DIM = 64
HID = 256
D = 8
NCORES = 8
BC = B // NCORES          # 8192 rows per core
BT = 512                  # batch tile (one fp32 PSUM bank of free dim)
NT = BC // BT             # 16 tiles per core
GRP = 4                   # tiles per group (head col-tiling width)
LOG_2PI = float(np.log(2.0 * np.pi))

F32 = mybir.dt.float32
BF16 = mybir.dt.bfloat16
RELU = mybir.ActivationFunctionType.Relu
EXP = mybir.ActivationFunctionType.Exp
ADD = mybir.AluOpType.add
MAX = mybir.AluOpType.max
MIN = mybir.AluOpType.min
MULT = mybir.AluOpType.mult

TRACE = False           # test.py flips this to get the NTFF profile
_NC_CACHE = {}


def _build_bass():
    nc = bacc.Bacc()

    xT = nc.declare_dram_parameter("xT", [IN_DIM, BC], BF16, isOutput=False)
    epsT = nc.declare_dram_parameter("epsT", [D, BC], F32, isOutput=False)
    w0 = nc.declare_dram_parameter("w0", [IN_DIM, HID], BF16, isOutput=False)
    w1 = nc.declare_dram_parameter("w1", [128, 2 * HID], BF16, isOutput=False)
    w2 = nc.declare_dram_parameter("w2", [128, 2 * HID], BF16, isOutput=False)
    wi = nc.declare_dram_parameter("wi", [128, D * 2 * HID], BF16, isOutput=False)
    wx = nc.declare_dram_parameter("wx", [D - 1, D * HID], BF16, isOutput=False)
    wh = nc.declare_dram_parameter("wh", [128, D * 2 * HID], BF16, isOutput=False)
    wo = nc.declare_dram_parameter("wo", [128, D * 2 * 2], BF16, isOutput=False)
    b0 = nc.declare_dram_parameter("b0", [128, 2], F32, isOutput=False)
    b1 = nc.declare_dram_parameter("b1", [128, 2], F32, isOutput=False)
    b2 = nc.declare_dram_parameter("b2", [128, 2], F32, isOutput=False)
    bi = nc.declare_dram_parameter("bi", [128, D * 2], F32, isOutput=False)
    bh = nc.declare_dram_parameter("bh", [128, D * 2], F32, isOutput=False)
    bo = nc.declare_dram_parameter("bo", [2, D], F32, isOutput=False)
    omT = nc.declare_dram_parameter("omT", [D, BC], F32, isOutput=True)
    osT = nc.declare_dram_parameter("osT", [D, BC], F32, isOutput=True)
    olT = nc.declare_dram_parameter("olT", [D, BC], F32, isOutput=True)

    with tile.TileContext(nc) as tc, ExitStack() as ctx:
        wp = ctx.enter_context(tc.tile_pool(name="w", bufs=1))
        hpool = ctx.enter_context(tc.tile_pool(name="h", bufs=NT // 2 + 1))
        mlpool = ctx.enter_context(tc.tile_pool(name="ml", bufs=NT + 2))
        xpool = ctx.enter_context(tc.tile_pool(name="xin", bufs=4))
        tpool = ctx.enter_context(tc.tile_pool(name="tr", bufs=2))
        epool = ctx.enter_context(tc.tile_pool(name="ep", bufs=3))
        opool = ctx.enter_context(tc.tile_pool(name="out", bufs=2))
        pspair = ctx.enter_context(tc.tile_pool(name="pspair", bufs=3, space="PSUM"))
        pshead = ctx.enter_context(tc.tile_pool(name="pshead", bufs=2, space="PSUM"))

        # ---- resident weights ----
        w0_s = wp.tile([IN_DIM, HID], BF16)
        nc.sync.dma_start(w0_s[:], w0[:])
        w1_s = wp.tile([128, 2, HID], BF16)
        nc.sync.dma_start(w1_s[:], w1[:].rearrange("p (k m) -> p k m", k=2))
        w2_s = wp.tile([128, 2, HID], BF16)
        nc.sync.dma_start(w2_s[:], w2[:].rearrange("p (k m) -> p k m", k=2))
        wi_s = wp.tile([128, D, 2, HID], BF16)
        nc.sync.dma_start(wi_s[:], wi[:].rearrange("p (i k m) -> p i k m", i=D, k=2))
        wx_s = wp.tile([D - 1, D, HID], BF16)
        nc.sync.dma_start(wx_s[:], wx[:].rearrange("j (i m) -> j i m", i=D))
        wh_s = wp.tile([128, D, 2, HID], BF16)
        nc.sync.dma_start(wh_s[:], wh[:].rearrange("p (i k m) -> p i k m", i=D, k=2))
        wo_s = wp.tile([128, D, 2, 2], BF16)
        nc.sync.dma_start(wo_s[:], wo[:].rearrange("p (i k c) -> p i k c", i=D, k=2))
        b0_s = wp.tile([128, 2], F32)
        nc.sync.dma_start(b0_s[:], b0[:])
        b1_s = wp.tile([128, 2], F32)
        nc.sync.dma_start(b1_s[:], b1[:])
        b2_s = wp.tile([128, 2], F32)
        nc.sync.dma_start(b2_s[:], b2[:])
        bi_s = wp.tile([128, D, 2], F32)
        nc.sync.dma_start(bi_s[:], bi[:].rearrange("p (i m) -> p i m", i=D))
        bh_s = wp.tile([128, D, 2], F32)
        nc.sync.dma_start(bh_s[:], bh[:].rearrange("p (i m) -> p i m", i=D))
        bo_s = wp.tile([2, D], F32)
        nc.sync.dma_start(bo_s[:], bo[:])

        def evac_act(dst, src, bias):
            nc.scalar.activation(dst, src, RELU, bias=bias)

        def evac_dve(dst, src, bias):
            nc.vector.tensor_scalar(dst, src, bias, 0.0, ADD, MAX)

        # A "pair tile" holds two batch tiles: SBUF [128, m(2), slot(2), BT];
        # PSUM pair tiles are [128, slot(2), BT] (2 banks).

        def layer_pairs(weight_col, bias_col, rhs_of, dst_of, evacs, n_k, corr=None):
            """One dense layer over a group of 2 pairs (4 tiles).

            weight_col(k, m) -> lhsT AP; rhs_of(j, k) -> moving AP for tile j;
            dst_of(p, m) -> evac destination AP; evacs[p] -> evac fn.
            """
            for m in range(2):
                pss = [
                    pspair.tile([128, 2, BT], F32, tag="pspair", name=f"ps{m}{p}")
                    for p in range(2)
                ]
                for k in range(n_k):
                    wv = weight_col(k, m)
                    for p in range(2):
                        for s in range(2):
                            nc.tensor.matmul(
                                pss[p][:, s, :], wv, rhs_of(2 * p + s, k),
                                start=(k == 0), stop=(k == n_k - 1 and corr is None),
                            )
                if corr is not None:
                    wfn, rhs_c = corr
                    wv = wfn(m)
                    for p in range(2):
                        for s in range(2):
                            nc.tensor.matmul(
                                pss[p][:, s, :], wv, rhs_c(2 * p + s),
                                start=False, stop=True,
                            )
                for p in range(2):
                    evacs[p](dst_of(p, m), pss[p][:], bias_col(m))

        def trunk_group(g, hps):
            """Trunk for tiles [4g..4g+4); writes h into hps[p] pair tiles."""
            t0 = GRP * g
            xts = []
            for j in range(GRP):
                xt = xpool.tile([IN_DIM, BT], BF16, tag="xt", name=f"xt{g}_{j}")
                nc.sync.dma_start(xt[:], xT[:, bass.ts(t0 + j, BT)])
                xts.append(xt)
            hp = [tpool.tile([128, 2, 2, BT], BF16, tag=f"hp{p}", name=f"hp{g}_{p}") for p in range(2)]
            layer_pairs(
                lambda k, m: w0_s[:, bass.ts(m, 128)],
                lambda m: b0_s[:, m : m + 1],
                lambda j, k: xts[j][:],
                lambda p, m: hp[p][:, m, :, :],
                [evac_act, evac_dve],
                n_k=1,
            )
            hq = [tpool.tile([128, 2, 2, BT], BF16, tag=f"hq{p}", name=f"hq{g}_{p}") for p in range(2)]
            layer_pairs(
                lambda k, m: w1_s[:, k, bass.ts(m, 128)],
                lambda m: b1_s[:, m : m + 1],
                lambda j, k: hp[j // 2][:, k, j % 2, :],
                lambda p, m: hq[p][:, m, :, :],
                [evac_dve, evac_act],
                n_k=2,
            )
            layer_pairs(
                lambda k, m: w2_s[:, k, bass.ts(m, 128)],
                lambda m: b2_s[:, m : m + 1],
                lambda j, k: hq[j // 2][:, k, j % 2, :],
                lambda p, m: hps[p][:, m, :, :],
                [evac_act, evac_dve],
                n_k=2,
            )

        def step_group(i, g, hps, mlts):
            """Step i for the 4 tiles of group g."""
            x1 = [tpool.tile([128, 2, 2, BT], BF16, tag=f"x1{p}", name=f"x1{g}_{p}") for p in range(2)]
            corr = None
            if i > 0:
                corr = (
                    lambda m: wx_s[0:i, i, bass.ts(m, 128)],
                    lambda j: mlts[j][0:i, :],
                )
            layer_pairs(
                lambda k, m: wi_s[:, i, k, bass.ts(m, 128)],
                lambda m: bi_s[:, i, m : m + 1],
                lambda j, k: hps[j // 2][:, k, j % 2, :],
                lambda p, m: x1[p][:, m, :, :],
                [evac_act, evac_dve],
                n_k=2,
                corr=corr,
            )
            x2 = [tpool.tile([128, 2, 2, BT], BF16, tag=f"x2{p}", name=f"x2{g}_{p}") for p in range(2)]
            layer_pairs(
                lambda k, m: wh_s[:, i, k, bass.ts(m, 128)],
                lambda m: bh_s[:, i, m : m + 1],
                lambda j, k: x1[j // 2][:, k, j % 2, :],
                lambda p, m: x2[p][:, m, :, :],
                [evac_dve, evac_act],
                n_k=2,
            )
            # head: the 4 tiles' M=2 matmuls run concurrently in distinct
            # PE column groups (tile_position), landing at psum partitions 32j.
            pso = pshead.tile([128, BT], F32, tag="pshead")
            for k in range(2):
                for j in range(GRP):
                    nc.tensor.matmul(
                        pso[32 * j : 32 * j + 2, :],
                        wo_s[:, i, k, :],
                        x2[j // 2][:, k, j % 2, :],
                        start=(k == 0), stop=(k == 1),
                        tile_position=(0, 32 * j),
                    )
            for j in range(GRP):
                sm = xpool.tile([2, BT], BF16, tag="sm", name=f"sm{g}_{j}")
                evac_act(sm[:], pso[32 * j : 32 * j + 2, :], bo_s[:, i : i + 1])
                nc.sync.dma_start(mlts[j][i : i + 1, :], sm[0:1, :])
                nc.sync.dma_start(mlts[j][32 + i : 32 + i + 1, :], sm[1:2, :])

        def epilogue(t, ml_t):
            et = epool.tile([D, BT], F32, tag="et")
            nc.sync.dma_start(et[:], epsT[:, bass.ts(t, BT)])
            mean_f = opool.tile([D, BT], F32, tag="mean_f")
            nc.vector.tensor_copy(mean_f[:], ml_t[0:D, :])
            ls = opool.tile([D, BT], F32, tag="ls")
            nc.vector.tensor_single_scalar(ls[:], ml_t[32 : 32 + D, :], 2.0, MIN)
            st = opool.tile([D, BT], F32, tag="st")
            nc.scalar.activation(st[:], ls[:], EXP)
            smp = opool.tile([D, BT], F32, tag="smp")
            nc.vector.tensor_mul(smp[:], st[:], et[:])
            smp2 = opool.tile([D, BT], F32, tag="smp2")
            nc.vector.tensor_add(smp2[:], smp[:], mean_f[:])
            sq = opool.tile([D, BT], F32, tag="sq")
            nc.vector.tensor_mul(sq[:], et[:], et[:])
            lp = opool.tile([D, BT], F32, tag="lp")
            nc.vector.tensor_scalar(lp[:], sq[:], -0.5, -0.5 * LOG_2PI, MULT, ADD)
            lp2 = opool.tile([D, BT], F32, tag="lp2")
            nc.vector.tensor_sub(lp2[:], lp[:], ls[:])
            nc.sync.dma_start(omT[:, bass.ts(t, BT)], mean_f[:])
            nc.sync.dma_start(osT[:, bass.ts(t, BT)], smp2[:])
            nc.sync.dma_start(olT[:, bass.ts(t, BT)], lp2[:])

        NG = NT // GRP  # 4 groups
        WAVEG = 2       # groups per wave
        for wv in range(NG // WAVEG):
            groups = list(range(wv * WAVEG, (wv + 1) * WAVEG))
            hs = {}
            mls = {}
            for g in groups:
                hs[g] = [
                    hpool.tile([128, 2, 2, BT], BF16, tag="h", name=f"h{g}_{p}")
                    for p in range(2)
                ]
                trunk_group(g, hs[g])
                mls[g] = [
                    mlpool.tile([32 + D, BT], BF16, tag="ml", name=f"ml{g}_{j}")
                    for j in range(GRP)
                ]
            for i in range(D):
                for g in groups:
                    step_group(i, g, hs[g], mls[g])
                    if i == D - 1:
                        for j in range(GRP):
                            epilogue(GRP * g + j, mls[g][j])

    nc.compile()
    return nc


def _get_nc():
    if "nc" not in _NC_CACHE:
        _NC_CACHE["nc"] = _build_bass()
    return _NC_CACHE["nc"]


def kernel(**inputs):
    import ml_dtypes

    bf16 = ml_dtypes.bfloat16
    inp = {k: np.ascontiguousarray(np.asarray(v, dtype=np.float32)) for k, v in inputs.items()}
    x = inp["inputs"]
    eps = inp["eps"]
    W_in, b_in = inp["W_in"], inp["b_in"]
    W_h, b_h = inp["W_h"], inp["b_h"]
    W_out, b_out = inp["W_out"], inp["b_out"]

    def cb(a):
        return np.ascontiguousarray(a.astype(bf16))

    c = np.ascontiguousarray
    shared = {
        "w0": cb(inp["sW0"]),
        "w1": cb(inp["sW1"].reshape(2, 128, HID).transpose(1, 0, 2).reshape(128, -1)),
        "w2": cb(inp["sW2"].reshape(2, 128, HID).transpose(1, 0, 2).reshape(128, -1)),
        "wi": cb(W_in[:, :HID, :].reshape(D, 2, 128, HID).transpose(2, 0, 1, 3).reshape(128, -1)),
        "wx": cb(W_in[:, HID:, :].transpose(1, 0, 2).reshape(D - 1, -1)),
        "wh": cb(W_h.reshape(D, 2, 128, HID).transpose(2, 0, 1, 3).reshape(128, -1)),
        "wo": cb(W_out.reshape(D, 2, 128, 2).transpose(2, 0, 1, 3).reshape(128, -1)),
        "b0": c(inp["sb0"].reshape(2, 128).T),
        "b1": c(inp["sb1"].reshape(2, 128).T),
        "b2": c(inp["sb2"].reshape(2, 128).T),
        "bi": c(b_in.reshape(D, 2, 128).transpose(2, 0, 1).reshape(128, -1)),
        "bh": c(b_h.reshape(D, 2, 128).transpose(2, 0, 1).reshape(128, -1)),
        "bo": c(b_out.T),
    }

    in_maps = []
    for core in range(NCORES):
        sl = slice(core * BC, (core + 1) * BC)
        m = dict(shared)
        m["xT"] = cb(x[sl].T)
        m["epsT"] = c(eps[sl].T)
        in_maps.append(m)

    nc = _get_nc()
    kw = {}
    if TRACE:
        import shutil

        shutil.rmtree("/tmp/ktrace", ignore_errors=True)
        os.makedirs("/tmp/ktrace", exist_ok=True)
        kw = dict(trace=True, trace_cores=[0], tmpdir="/tmp/ktrace")
    res = run_bass_kernel_spmd(nc, in_maps, list(range(NCORES)), **kw)
    if TRACE:
        print(f"HW exec time: {res.exec_time_ns} ns")

    out_mean = np.concatenate([res.results[i]["omT"].T for i in range(NCORES)], axis=0)
    out_sample = np.concatenate([res.results[i]["osT"].T for i in range(NCORES)], axis=0)
    out_logp = np.concatenate([res.results[i]["olT"].T for i in range(NCORES)], axis=0)
    return out_mean, out_sample, out_logp


# revision 15
# speedup vs baseline: 1.2473x; 1.0207x over previous
"""Trainium2 Bass kernel for the autoregressive policy head (nn_ADM_6511170421537).

Structure (per core, pure data parallelism over 8 cores):
  trunk:  h = relu(x@sW0+b) -> relu(@sW1+b) -> relu(@sW2+b)          [B,256]
  steps i=0..7 (sequential in i, batch-parallel):
      x1 = relu(h@W_in[i][:256] + means[:i]@W_in[i][256:256+i] + b_in[i])
      x2 = relu(x1@W_h[i] + b_h[i])
      (mean_i, ls_i) = relu(x2@W_out[i] + b_out[i])
  epilogue (batched over the 8 steps, fp32):
      log_std = min(ls, 2);  std = exp(log_std)
      sample  = mean + std*eps
      logp    = -0.5*eps^2 - log_std - 0.5*log(2pi)   (== reference algebra)

Layout: feature-major on chip ([features->partitions, batch->free]); the host
transposes inputs/eps/outputs so every DMA moves contiguous lines.  Matmuls
run in bf16 (PSUM accumulates fp32), epilogue math in fp32.

Perf structure: batch tiles are processed in GROUPS of 4 (two PAIRS).  All
matmuls sharing a stationary operand are emitted back-to-back (weight-load
hides in the streaming of the previous matmul), a pair shares one 2-bank
PSUM tile so each PSUM->SBUF evacuation covers 2 tiles in one op (FD=1024),
the tiny M=2 head matmuls of the 4 tiles in a group run CONCURRENTLY in
disjoint PE column groups, and the small K=i "autoregressive correction"
matmuls of a pair run concurrently in disjoint PE row groups (the means are
DMA-scattered into per-slot 32-partition bands).
"""

import os

os.environ.setdefault("MYCRO_LOCAL_CACHE", "1")

import numpy as np
from contextlib import ExitStack

import concourse.bass as bass
import concourse.bacc as bacc
import concourse.mybir as mybir
import concourse.tile as tile
from concourse.bass_utils import run_bass_kernel_spmd

# ---- problem constants (hardcoded; kernel.py must be self-contained) ----
B = 65536
IN_DIM = 64
HID = 256
D = 8
NCORES = 8
BC = B // NCORES          # 8192 rows per core
BT = 512                  # batch tile (one fp32 PSUM bank of free dim)
NT = BC // BT             # 16 tiles per core
GRP = 4                   # tiles per group (head col-tiling width)
LOG_2PI = float(np.log(2.0 * np.pi))

F32 = mybir.dt.float32
BF16 = mybir.dt.bfloat16
RELU = mybir.ActivationFunctionType.Relu
EXP = mybir.ActivationFunctionType.Exp
ADD = mybir.AluOpType.add
MAX = mybir.AluOpType.max
MIN = mybir.AluOpType.min
MULT = mybir.AluOpType.mult

TRACE = False           # test.py flips this to get the NTFF profile
_NC_CACHE = {}


def _build_bass():
    nc = bacc.Bacc()

    xT = nc.declare_dram_parameter("xT", [IN_DIM, BC], BF16, isOutput=False)
    epsT = nc.declare_dram_parameter("epsT", [D, BC], F32, isOutput=False)
    w0 = nc.declare_dram_parameter("w0", [IN_DIM, HID], BF16, isOutput=False)
    w1 = nc.declare_dram_parameter("w1", [128, 2 * HID], BF16, isOutput=False)
    w2 = nc.declare_dram_parameter("w2", [128, 2 * HID], BF16, isOutput=False)
    wi = nc.declare_dram_parameter("wi", [128, D * 2 * HID], BF16, isOutput=False)
    # wx2: correction rows replicated in partition bands 0-6 and 32-38 so a
    # pair's two correction matmuls can run in disjoint PE row groups.
    wx2 = nc.declare_dram_parameter("wx2", [64, D * 2 * 128], BF16, isOutput=False)
    wh = nc.declare_dram_parameter("wh", [128, D * 2 * HID], BF16, isOutput=False)
    wo = nc.declare_dram_parameter("wo", [128, D * 2 * 2], BF16, isOutput=False)
    b0 = nc.declare_dram_parameter("b0", [128, 2], F32, isOutput=False)
    b1 = nc.declare_dram_parameter("b1", [128, 2], F32, isOutput=False)
    b2 = nc.declare_dram_parameter("b2", [128, 2], F32, isOutput=False)
    bi = nc.declare_dram_parameter("bi", [128, D * 2], F32, isOutput=False)
    bh = nc.declare_dram_parameter("bh", [128, D * 2], F32, isOutput=False)
    bo = nc.declare_dram_parameter("bo", [2, D], F32, isOutput=False)
    omT = nc.declare_dram_parameter("omT", [D, BC], F32, isOutput=True)
    osT = nc.declare_dram_parameter("osT", [D, BC], F32, isOutput=True)
    olT = nc.declare_dram_parameter("olT", [D, BC], F32, isOutput=True)

    with tile.TileContext(nc) as tc, ExitStack() as ctx:
        wp = ctx.enter_context(tc.tile_pool(name="w", bufs=1))
        hpool = ctx.enter_context(tc.tile_pool(name="h", bufs=NT // 2 + 1))
        mlpool = ctx.enter_context(tc.tile_pool(name="ml", bufs=NT + 2))
        xpool = ctx.enter_context(tc.tile_pool(name="xin", bufs=4))
        xtpool = ctx.enter_context(tc.tile_pool(name="xtp", bufs=NT))
        tpool = ctx.enter_context(tc.tile_pool(name="tr", bufs=2))
        epool = ctx.enter_context(tc.tile_pool(name="ep", bufs=2))
        opool = ctx.enter_context(tc.tile_pool(name="out", bufs=1))
        pspair = ctx.enter_context(tc.tile_pool(name="pspair", bufs=3, space="PSUM"))
        pshead = ctx.enter_context(tc.tile_pool(name="pshead", bufs=2, space="PSUM"))

        # ---- resident weights ----
        w0_s = wp.tile([IN_DIM, HID], BF16)
        nc.sync.dma_start(w0_s[:], w0[:])
        w1_s = wp.tile([128, 2, HID], BF16)
        nc.sync.dma_start(w1_s[:], w1[:].rearrange("p (k m) -> p k m", k=2))
        w2_s = wp.tile([128, 2, HID], BF16)
        nc.sync.dma_start(w2_s[:], w2[:].rearrange("p (k m) -> p k m", k=2))
        wi_s = wp.tile([128, D, 2, HID], BF16)
        nc.sync.dma_start(wi_s[:], wi[:].rearrange("p (i k m) -> p i k m", i=D, k=2))
        wx2_s = wp.tile([64, D, 2, 128], BF16)
        nc.sync.dma_start(wx2_s[:], wx2[:].rearrange("j (i m c) -> j i m c", i=D, m=2))
        wh_s = wp.tile([128, D, 2, HID], BF16)
        nc.sync.dma_start(wh_s[:], wh[:].rearrange("p (i k m) -> p i k m", i=D, k=2))
        wo_s = wp.tile([128, D, 2, 2], BF16)
        nc.sync.dma_start(wo_s[:], wo[:].rearrange("p (i k c) -> p i k c", i=D, k=2))
        b0_s = wp.tile([128, 2], F32)
        nc.sync.dma_start(b0_s[:], b0[:])
        b1_s = wp.tile([128, 2], F32)
        nc.sync.dma_start(b1_s[:], b1[:])
        b2_s = wp.tile([128, 2], F32)
        nc.sync.dma_start(b2_s[:], b2[:])
        bi_s = wp.tile([128, D, 2], F32)
        nc.sync.dma_start(bi_s[:], bi[:].rearrange("p (i m) -> p i m", i=D))
        bh_s = wp.tile([128, D, 2], F32)
        nc.sync.dma_start(bh_s[:], bh[:].rearrange("p (i m) -> p i m", i=D))
        bo_s = wp.tile([2, D], F32)
        nc.sync.dma_start(bo_s[:], bo[:])

        # prefetch ALL input tiles up front (tiny: 16 x 1KB/partition-line)
        xts = []
        for t in range(NT):
            xt = xtpool.tile([IN_DIM, BT], BF16, tag="xt", name=f"xt{t}")
            nc.sync.dma_start(xt[:], xT[:, bass.ts(t, BT)])
            xts.append(xt)

        def evac_act(dst, src, bias):
            nc.scalar.activation(dst, src, RELU, bias=bias)

        def evac_dve(dst, src, bias):
            nc.vector.tensor_scalar(dst, src, bias, 0.0, ADD, MAX)

        # A "pair tile" holds two batch tiles: SBUF [128, m(2), slot(2), BT];
        # PSUM pair tiles are [128, slot(2), BT] (2 banks).

        def layer_pairs(weight_col, bias_col, rhs_of, dst_of, evacs, n_k, corr_i=0,
                        mlps=None):
            """One dense layer over a group of 2 pairs (4 tiles)."""
            for m in range(2):
                pss = [
                    pspair.tile([128, 2, BT], F32, tag="pspair", name=f"ps{m}{p}")
                    for p in range(2)
                ]
                for k in range(n_k):
                    wv = weight_col(k, m)
                    for p in range(2):
                        for s in range(2):
                            nc.tensor.matmul(
                                pss[p][:, s, :], wv, rhs_of(2 * p + s, k),
                                start=(k == 0), stop=(k == n_k - 1 and corr_i == 0),
                            )
                if corr_i > 0:
                    i = corr_i
                    for p in range(2):
                        for s in range(2):
                            # slot s reads its means band at partitions 32s;
                            # the two slots run in disjoint PE row groups.
                            nc.tensor.matmul(
                                pss[p][:, s, :],
                                wx2_s[32 * s : 32 * s + i, i, m, :],
                                mlps[p][32 * s : 32 * s + i, :],
                                start=False, stop=True,
                                tile_position=(32 * s, 0),
                            )
                for p in range(2):
                    evacs[p](dst_of(p, m), pss[p][:], bias_col(m))

        def trunk_group(g, hps):
            """Trunk for tiles [4g..4g+4); writes h into hps[p] pair tiles."""
            t0 = GRP * g
            hp = [tpool.tile([128, 2, 2, BT], BF16, tag=f"hp{p}", name=f"hp{g}_{p}", bufs=1) for p in range(2)]
            layer_pairs(
                lambda k, m: w0_s[:, bass.ts(m, 128)],
                lambda m: b0_s[:, m : m + 1],
                lambda j, k: xts[t0 + j][:],
                lambda p, m: hp[p][:, m, :, :],
                [evac_act, evac_dve],
                n_k=1,
            )
            hq = [tpool.tile([128, 2, 2, BT], BF16, tag=f"hq{p}", name=f"hq{g}_{p}", bufs=1) for p in range(2)]
            layer_pairs(
                lambda k, m: w1_s[:, k, bass.ts(m, 128)],
                lambda m: b1_s[:, m : m + 1],
                lambda j, k: hp[j // 2][:, k, j % 2, :],
                lambda p, m: hq[p][:, m, :, :],
                [evac_dve, evac_act],
                n_k=2,
            )
            layer_pairs(
                lambda k, m: w2_s[:, k, bass.ts(m, 128)],
                lambda m: b2_s[:, m : m + 1],
                lambda j, k: hq[j // 2][:, k, j % 2, :],
                lambda p, m: hps[p][:, m, :, :],
                [evac_act, evac_dve],
                n_k=2,
            )

        def step_group(i, g, hps, mlps, lsps):
            """Step i for the 4 tiles of group g."""
            x1 = [tpool.tile([128, 2, 2, BT], BF16, tag=f"x1{p}", name=f"x1{g}_{p}") for p in range(2)]
            layer_pairs(
                lambda k, m: wi_s[:, i, k, bass.ts(m, 128)],
                lambda m: bi_s[:, i, m : m + 1],
                lambda j, k: hps[j // 2][:, k, j % 2, :],
                lambda p, m: x1[p][:, m, :, :],
                [evac_act, evac_dve],
                n_k=2,
                corr_i=i,
                mlps=mlps,
            )
            x2 = [tpool.tile([128, 2, 2, BT], BF16, tag=f"x2{p}", name=f"x2{g}_{p}") for p in range(2)]
            layer_pairs(
                lambda k, m: wh_s[:, i, k, bass.ts(m, 128)],
                lambda m: bh_s[:, i, m : m + 1],
                lambda j, k: x1[j // 2][:, k, j % 2, :],
                lambda p, m: x2[p][:, m, :, :],
                [evac_dve, evac_act],
                n_k=2,
            )
            # head: the 4 tiles' M=2 matmuls run concurrently in distinct
            # PE column groups (tile_position), landing at psum partitions 32j.
            pso = pshead.tile([128, BT], F32, tag="pshead")
            for k in range(2):
                for j in range(GRP):
                    nc.tensor.matmul(
                        pso[32 * j : 32 * j + 2, :],
                        wo_s[:, i, k, :],
                        x2[j // 2][:, k, j % 2, :],
                        start=(k == 0), stop=(k == 1),
                        tile_position=(0, 32 * j),
                    )
            for j in range(GRP):
                p, s = j // 2, j % 2
                sm = xpool.tile([2, BT], BF16, tag="sm", name=f"sm{g}_{j}")
                evac = evac_act if j % 2 == 0 else evac_dve
                evac(sm[:], pso[32 * j : 32 * j + 2, :], bo_s[:, i : i + 1])
                nc.sync.dma_start(mlps[p][32 * s + i : 32 * s + i + 1, :], sm[0:1, :])
                nc.sync.dma_start(lsps[p][i : i + 1, s, :], sm[1:2, :])

        def epilogue_pair(g, p, mlp, lsp):
            """Epilogue for pair p of group g (two tiles, FD=1024 ops)."""
            t0 = GRP * g + 2 * p
            et = epool.tile([D, 2, BT], F32, tag="et")
            nc.sync.dma_start(et[:], epsT[:, bass.ts(t0 // 2, 2 * BT)])
            mean_f = opool.tile([D, 2, BT], F32, tag="mean_f")
            nc.vector.tensor_copy(mean_f[:, 0, :], mlp[0:D, :])
            nc.vector.tensor_copy(mean_f[:, 1, :], mlp[32 : 32 + D, :])
            ls = opool.tile([D, 2, BT], F32, tag="ls")
            nc.vector.tensor_single_scalar(ls[:], lsp[:], 2.0, MIN)
            st = opool.tile([D, 2, BT], F32, tag="st")
            nc.scalar.activation(st[:], ls[:], EXP)
            nc.sync.dma_start(omT[:, bass.ts(t0 // 2, 2 * BT)], mean_f[:])
            # elementwise in-place: DVE writes trail reads through the pipe
            nc.vector.tensor_mul(st[:], st[:], et[:])            # st*eps
            nc.vector.tensor_add(st[:], st[:], mean_f[:])        # sample
            nc.sync.dma_start(osT[:, bass.ts(t0 // 2, 2 * BT)], st[:])
            nc.vector.tensor_mul(et[:], et[:], et[:])            # eps^2
            nc.vector.tensor_scalar(et[:], et[:], -0.5, -0.5 * LOG_2PI, MULT, ADD)
            nc.vector.tensor_sub(et[:], et[:], ls[:])            # logp
            nc.sync.dma_start(olT[:, bass.ts(t0 // 2, 2 * BT)], et[:])

        NG = NT // GRP  # 4 groups
        WAVEG = 2       # groups per wave
        state = {}
        for g in range(NG):
            state[g] = dict(
                h=[hpool.tile([128, 2, 2, BT], BF16, tag="h", name=f"h{g}_{p}") for p in range(2)],
                mlp=[mlpool.tile([40, BT], BF16, tag="mlp", name=f"mlp{g}_{p}") for p in range(2)],
                lsp=[mlpool.tile([D, 2, BT], BF16, tag="lsp", name=f"lsp{g}_{p}") for p in range(2)],
            )

        for wv in range(NG // WAVEG):
            groups = list(range(wv * WAVEG, (wv + 1) * WAVEG))
            if wv == 0:
                for g in groups:
                    trunk_group(g, state[g]["h"])
            for i in range(D):
                for g in groups:
                    st_ = state[g]
                    step_group(i, g, st_["h"], st_["mlp"], st_["lsp"])
                # emit next wave's trunk early so the PE has work across
                # the wave boundary
                if i == D - 2 and wv + 1 < NG // WAVEG:
                    for g2 in range((wv + 1) * WAVEG, (wv + 2) * WAVEG):
                        trunk_group(g2, state[g2]["h"])
                if i == D - 1:
                    for g in groups:
                        for p in range(2):
                            epilogue_pair(g, p, state[g]["mlp"][p], state[g]["lsp"][p])

    nc.compile()
    return nc


def _get_nc():
    if "nc" not in _NC_CACHE:
        _NC_CACHE["nc"] = _build_bass()
    return _NC_CACHE["nc"]


def kernel(**inputs):
    import ml_dtypes

    bf16 = ml_dtypes.bfloat16
    inp = {k: np.ascontiguousarray(np.asarray(v, dtype=np.float32)) for k, v in inputs.items()}
    x = inp["inputs"]
    eps = inp["eps"]
    W_in, b_in = inp["W_in"], inp["b_in"]
    W_h, b_h = inp["W_h"], inp["b_h"]
    W_out, b_out = inp["W_out"], inp["b_out"]

    def cb(a):
        return np.ascontiguousarray(a.astype(bf16))

    c = np.ascontiguousarray

    # wx2: [64, D, 2, 128] with correction rows at partition bands 0 and 32
    wx2 = np.zeros((64, D, 2, 128), np.float32)
    ext = W_in[:, HID:, :]  # [D, 7, 256]
    for s in range(2):
        for j in range(D - 1):
            for m in range(2):
                wx2[32 * s + j, :, m, :] = ext[:, j, 128 * m : 128 * (m + 1)]

    shared = {
        "w0": cb(inp["sW0"]),
        "w1": cb(inp["sW1"].reshape(2, 128, HID).transpose(1, 0, 2).reshape(128, -1)),
        "w2": cb(inp["sW2"].reshape(2, 128, HID).transpose(1, 0, 2).reshape(128, -1)),
        "wi": cb(W_in[:, :HID, :].reshape(D, 2, 128, HID).transpose(2, 0, 1, 3).reshape(128, -1)),
        "wx2": cb(wx2.reshape(64, -1)),
        "wh": cb(W_h.reshape(D, 2, 128, HID).transpose(2, 0, 1, 3).reshape(128, -1)),
        "wo": cb(W_out.reshape(D, 2, 128, 2).transpose(2, 0, 1, 3).reshape(128, -1)),
        "b0": c(inp["sb0"].reshape(2, 128).T),
        "b1": c(inp["sb1"].reshape(2, 128).T),
        "b2": c(inp["sb2"].reshape(2, 128).T),
        "bi": c(b_in.reshape(D, 2, 128).transpose(2, 0, 1).reshape(128, -1)),
        "bh": c(b_h.reshape(D, 2, 128).transpose(2, 0, 1).reshape(128, -1)),
        "bo": c(b_out.T),
    }

    in_maps = []
    for core in range(NCORES):
        sl = slice(core * BC, (core + 1) * BC)
        m = dict(shared)
        m["xT"] = cb(x[sl].T)
        m["epsT"] = c(eps[sl].T)
        in_maps.append(m)

    nc = _get_nc()
    kw = {}
    if TRACE:
        import shutil

        shutil.rmtree("/tmp/ktrace", ignore_errors=True)
        os.makedirs("/tmp/ktrace", exist_ok=True)
        kw = dict(trace=True, trace_cores=[0], tmpdir="/tmp/ktrace")
    res = run_bass_kernel_spmd(nc, in_maps, list(range(NCORES)), **kw)
    if TRACE:
        print(f"HW exec time: {res.exec_time_ns} ns")

    out_mean = np.concatenate([res.results[i]["omT"].T for i in range(NCORES)], axis=0)
    out_sample = np.concatenate([res.results[i]["osT"].T for i in range(NCORES)], axis=0)
    out_logp = np.concatenate([res.results[i]["olT"].T for i in range(NCORES)], axis=0)
    return out_mean, out_sample, out_logp


# revision 16
# speedup vs baseline: 1.2928x; 1.0365x over previous
"""Trainium2 Bass kernel for the autoregressive policy head (nn_ADM_6511170421537).

Structure (per core, pure data parallelism over 8 cores):
  trunk:  h = relu(x@sW0+b) -> relu(@sW1+b) -> relu(@sW2+b)          [B,256]
  steps i=0..7 (sequential in i, batch-parallel):
      x1 = relu(h@W_in[i][:256] + means[:i]@W_in[i][256:256+i] + b_in[i])
      x2 = relu(x1@W_h[i] + b_h[i])
      (mean_i, ls_i) = relu(x2@W_out[i] + b_out[i])
  epilogue (batched over the 8 steps, fp32):
      log_std = min(ls, 2);  std = exp(log_std)
      sample  = mean + std*eps
      logp    = -0.5*eps^2 - log_std - 0.5*log(2pi)   (== reference algebra)

Layout: feature-major on chip ([features->partitions, batch->free]); the host
transposes inputs/eps/outputs so every DMA moves contiguous lines.  Matmuls
run in bf16 (PSUM accumulates fp32), epilogue math in fp32.

Perf structure: batch tiles are processed in GROUPS of 4 (two PAIRS).  All
matmuls sharing a stationary operand are emitted back-to-back (weight-load
hides in the streaming of the previous matmul), a pair shares one 2-bank
PSUM tile so each PSUM->SBUF evacuation covers 2 tiles in one op (FD=1024),
the tiny M=2 head matmuls of the 4 tiles in a group run CONCURRENTLY in
disjoint PE column groups, and the small K=i "autoregressive correction"
matmuls of a pair run concurrently in disjoint PE row groups (the means are
DMA-scattered into per-slot 32-partition bands).
"""

import os

os.environ.setdefault("MYCRO_LOCAL_CACHE", "1")

import numpy as np
from contextlib import ExitStack

import concourse.bass as bass
import concourse.bacc as bacc
import concourse.mybir as mybir
import concourse.tile as tile
from concourse.bass_utils import run_bass_kernel_spmd

# ---- problem constants (hardcoded; kernel.py must be self-contained) ----
B = 65536
IN_DIM = 64
HID = 256
D = 8
NCORES = 8
BC = B // NCORES          # 8192 rows per core
BT = 512                  # batch tile (one fp32 PSUM bank of free dim)
NT = BC // BT             # 16 tiles per core
GRP = 4                   # tiles per group (head col-tiling width)
LOG_2PI = float(np.log(2.0 * np.pi))

F32 = mybir.dt.float32
BF16 = mybir.dt.bfloat16
RELU = mybir.ActivationFunctionType.Relu
EXP = mybir.ActivationFunctionType.Exp
ADD = mybir.AluOpType.add
MAX = mybir.AluOpType.max
MIN = mybir.AluOpType.min
MULT = mybir.AluOpType.mult

TRACE = False           # test.py flips this to get the NTFF profile
_NC_CACHE = {}


def _build_bass():
    nc = bacc.Bacc()

    xT = nc.declare_dram_parameter("xT", [IN_DIM, BC], BF16, isOutput=False)
    epsT = nc.declare_dram_parameter("epsT", [D, BC], F32, isOutput=False)
    w0 = nc.declare_dram_parameter("w0", [IN_DIM, HID], BF16, isOutput=False)
    w1 = nc.declare_dram_parameter("w1", [128, 2 * HID], BF16, isOutput=False)
    w2 = nc.declare_dram_parameter("w2", [128, 2 * HID], BF16, isOutput=False)
    wi = nc.declare_dram_parameter("wi", [128, D * 2 * HID], BF16, isOutput=False)
    # wx2: correction rows replicated in partition bands 0-6 and 32-38 so a
    # pair's two correction matmuls can run in disjoint PE row groups.
    wx2 = nc.declare_dram_parameter("wx2", [64, D * 2 * 128], BF16, isOutput=False)
    wh = nc.declare_dram_parameter("wh", [128, D * 2 * HID], BF16, isOutput=False)
    wo = nc.declare_dram_parameter("wo", [128, D * 2 * 2], BF16, isOutput=False)
    b0 = nc.declare_dram_parameter("b0", [128, 2], F32, isOutput=False)
    b1 = nc.declare_dram_parameter("b1", [128, 2], F32, isOutput=False)
    b2 = nc.declare_dram_parameter("b2", [128, 2], F32, isOutput=False)
    bi = nc.declare_dram_parameter("bi", [128, D * 2], F32, isOutput=False)
    bh = nc.declare_dram_parameter("bh", [128, D * 2], F32, isOutput=False)
    bo = nc.declare_dram_parameter("bo", [128, D], F32, isOutput=False)
    omT = nc.declare_dram_parameter("omT", [D, BC], F32, isOutput=True)
    osT = nc.declare_dram_parameter("osT", [D, BC], F32, isOutput=True)
    olT = nc.declare_dram_parameter("olT", [D, BC], F32, isOutput=True)

    with tile.TileContext(nc) as tc, ExitStack() as ctx:
        wp = ctx.enter_context(tc.tile_pool(name="w", bufs=1))
        hpool = ctx.enter_context(tc.tile_pool(name="h", bufs=NT // 2 + 1))
        mlpool = ctx.enter_context(tc.tile_pool(name="ml", bufs=NT + 2))
        xpool = ctx.enter_context(tc.tile_pool(name="xin", bufs=4))
        xtpool = ctx.enter_context(tc.tile_pool(name="xtp", bufs=NT))
        tpool = ctx.enter_context(tc.tile_pool(name="tr", bufs=2))
        epool = ctx.enter_context(tc.tile_pool(name="ep", bufs=2))
        opool = ctx.enter_context(tc.tile_pool(name="out", bufs=1))
        pspair = ctx.enter_context(tc.tile_pool(name="pspair", bufs=3, space="PSUM"))
        pshead = ctx.enter_context(tc.tile_pool(name="pshead", bufs=2, space="PSUM"))

        # ---- resident weights ----
        w0_s = wp.tile([IN_DIM, HID], BF16)
        nc.sync.dma_start(w0_s[:], w0[:])
        w1_s = wp.tile([128, 2, HID], BF16)
        nc.sync.dma_start(w1_s[:], w1[:].rearrange("p (k m) -> p k m", k=2))
        w2_s = wp.tile([128, 2, HID], BF16)
        nc.sync.dma_start(w2_s[:], w2[:].rearrange("p (k m) -> p k m", k=2))
        wi_s = wp.tile([128, D, 2, HID], BF16)
        nc.sync.dma_start(wi_s[:], wi[:].rearrange("p (i k m) -> p i k m", i=D, k=2))
        wx2_s = wp.tile([64, D, 2, 128], BF16)
        nc.sync.dma_start(wx2_s[:], wx2[:].rearrange("j (i m c) -> j i m c", i=D, m=2))
        wh_s = wp.tile([128, D, 2, HID], BF16)
        nc.sync.dma_start(wh_s[:], wh[:].rearrange("p (i k m) -> p i k m", i=D, k=2))
        wo_s = wp.tile([128, D, 2, 2], BF16)
        nc.sync.dma_start(wo_s[:], wo[:].rearrange("p (i k c) -> p i k c", i=D, k=2))
        b0_s = wp.tile([128, 2], F32)
        nc.sync.dma_start(b0_s[:], b0[:])
        b1_s = wp.tile([128, 2], F32)
        nc.sync.dma_start(b1_s[:], b1[:])
        b2_s = wp.tile([128, 2], F32)
        nc.sync.dma_start(b2_s[:], b2[:])
        bi_s = wp.tile([128, D, 2], F32)
        nc.sync.dma_start(bi_s[:], bi[:].rearrange("p (i m) -> p i m", i=D))
        bh_s = wp.tile([128, D, 2], F32)
        nc.sync.dma_start(bh_s[:], bh[:].rearrange("p (i m) -> p i m", i=D))
        bo_s = wp.tile([128, D], F32)
        nc.sync.dma_start(bo_s[:], bo[:])

        # prefetch ALL input tiles up front (tiny: 16 x 1KB/partition-line)
        xts = []
        for t in range(NT):
            xt = xtpool.tile([IN_DIM, BT], BF16, tag="xt", name=f"xt{t}")
            nc.sync.dma_start(xt[:], xT[:, bass.ts(t, BT)])
            xts.append(xt)

        def evac_act(dst, src, bias):
            nc.scalar.activation(dst, src, RELU, bias=bias)

        def evac_dve(dst, src, bias):
            nc.vector.tensor_scalar(dst, src, bias, 0.0, ADD, MAX)

        # A "pair tile" holds two batch tiles: SBUF [128, m(2), slot(2), BT];
        # PSUM pair tiles are [128, slot(2), BT] (2 banks).

        def layer_pairs(weight_col, bias_col, rhs_of, dst_of, evacs, n_k, corr_i=0,
                        mlps=None):
            """One dense layer over a group of 2 pairs (4 tiles)."""
            for m in range(2):
                pss = [
                    pspair.tile([128, 2, BT], F32, tag="pspair", name=f"ps{m}{p}")
                    for p in range(2)
                ]
                for k in range(n_k):
                    wv = weight_col(k, m)
                    for p in range(2):
                        for s in range(2):
                            nc.tensor.matmul(
                                pss[p][:, s, :], wv, rhs_of(2 * p + s, k),
                                start=(k == 0), stop=(k == n_k - 1 and corr_i == 0),
                            )
                if corr_i > 0:
                    i = corr_i
                    for p in range(2):
                        for s in range(2):
                            # slot s reads its means band at partitions 32s;
                            # the two slots run in disjoint PE row groups.
                            nc.tensor.matmul(
                                pss[p][:, s, :],
                                wx2_s[32 * s : 32 * s + i, i, m, :],
                                mlps[p][32 * s : 32 * s + i, :],
                                start=False, stop=True,
                                tile_position=(32 * s, 0),
                            )
                for p in range(2):
                    evacs[p](dst_of(p, m), pss[p][:], bias_col(m))

        def trunk_group(g, hps):
            """Trunk for tiles [4g..4g+4); writes h into hps[p] pair tiles."""
            t0 = GRP * g
            hp = [tpool.tile([128, 2, 2, BT], BF16, tag=f"hp{p}", name=f"hp{g}_{p}", bufs=1) for p in range(2)]
            layer_pairs(
                lambda k, m: w0_s[:, bass.ts(m, 128)],
                lambda m: b0_s[:, m : m + 1],
                lambda j, k: xts[t0 + j][:],
                lambda p, m: hp[p][:, m, :, :],
                [evac_act, evac_dve],
                n_k=1,
            )
            hq = [tpool.tile([128, 2, 2, BT], BF16, tag=f"hq{p}", name=f"hq{g}_{p}", bufs=1) for p in range(2)]
            layer_pairs(
                lambda k, m: w1_s[:, k, bass.ts(m, 128)],
                lambda m: b1_s[:, m : m + 1],
                lambda j, k: hp[j // 2][:, k, j % 2, :],
                lambda p, m: hq[p][:, m, :, :],
                [evac_dve, evac_act],
                n_k=2,
            )
            layer_pairs(
                lambda k, m: w2_s[:, k, bass.ts(m, 128)],
                lambda m: b2_s[:, m : m + 1],
                lambda j, k: hq[j // 2][:, k, j % 2, :],
                lambda p, m: hps[p][:, m, :, :],
                [evac_act, evac_dve],
                n_k=2,
            )

        def step_group(i, g, hps, mlps, lsps):
            """Step i for the 4 tiles of group g."""
            x1 = [tpool.tile([128, 2, 2, BT], BF16, tag=f"x1{p}", name=f"x1{g}_{p}") for p in range(2)]
            layer_pairs(
                lambda k, m: wi_s[:, i, k, bass.ts(m, 128)],
                lambda m: bi_s[:, i, m : m + 1],
                lambda j, k: hps[j // 2][:, k, j % 2, :],
                lambda p, m: x1[p][:, m, :, :],
                [evac_act, evac_dve],
                n_k=2,
                corr_i=i,
                mlps=mlps,
            )
            x2 = [tpool.tile([128, 2, 2, BT], BF16, tag=f"x2{p}", name=f"x2{g}_{p}") for p in range(2)]
            layer_pairs(
                lambda k, m: wh_s[:, i, k, bass.ts(m, 128)],
                lambda m: bh_s[:, i, m : m + 1],
                lambda j, k: x1[j // 2][:, k, j % 2, :],
                lambda p, m: x2[p][:, m, :, :],
                [evac_dve, evac_act],
                n_k=2,
            )
            # head: the 4 tiles' M=2 matmuls run concurrently in distinct
            # PE column groups (tile_position), landing at psum partitions 32j.
            pso = pshead.tile([128, BT], F32, tag="pshead")
            for k in range(2):
                for j in range(GRP):
                    nc.tensor.matmul(
                        pso[32 * j : 32 * j + 2, :],
                        wo_s[:, i, k, :],
                        x2[j // 2][:, k, j % 2, :],
                        start=(k == 0), stop=(k == 1),
                        tile_position=(0, 32 * j),
                    )
            sm = xpool.tile([128, BT], BF16, tag="sm", name=f"sm{g}_{i}")
            evac_act(sm[0:98, :], pso[0:98, :], bo_s[0:98, i : i + 1])
            for j in range(GRP):
                p, s = j // 2, j % 2
                nc.sync.dma_start(
                    mlps[p][32 * s + i : 32 * s + i + 1, :], sm[32 * j : 32 * j + 1, :]
                )
                nc.sync.dma_start(
                    lsps[p][i : i + 1, s, :], sm[32 * j + 1 : 32 * j + 2, :]
                )

        def epilogue_pair(g, p, mlp, lsp):
            """Epilogue for pair p of group g (two tiles, FD=1024 ops)."""
            t0 = GRP * g + 2 * p
            et = epool.tile([D, 2, BT], F32, tag="et")
            nc.sync.dma_start(et[:], epsT[:, bass.ts(t0 // 2, 2 * BT)])
            mean_f = opool.tile([D, 2, BT], F32, tag="mean_f")
            nc.vector.tensor_copy(mean_f[:, 0, :], mlp[0:D, :])
            nc.vector.tensor_copy(mean_f[:, 1, :], mlp[32 : 32 + D, :])
            ls = opool.tile([D, 2, BT], F32, tag="ls")
            nc.vector.tensor_single_scalar(ls[:], lsp[:], 2.0, MIN)
            st = opool.tile([D, 2, BT], F32, tag="st")
            nc.scalar.activation(st[:], ls[:], EXP)
            nc.sync.dma_start(omT[:, bass.ts(t0 // 2, 2 * BT)], mean_f[:])
            # elementwise in-place: DVE writes trail reads through the pipe
            nc.vector.tensor_mul(st[:], st[:], et[:])            # st*eps
            nc.vector.tensor_add(st[:], st[:], mean_f[:])        # sample
            nc.sync.dma_start(osT[:, bass.ts(t0 // 2, 2 * BT)], st[:])
            nc.vector.tensor_mul(et[:], et[:], et[:])            # eps^2
            nc.vector.tensor_scalar(et[:], et[:], -0.5, -0.5 * LOG_2PI, MULT, ADD)
            nc.vector.tensor_sub(et[:], et[:], ls[:])            # logp
            nc.sync.dma_start(olT[:, bass.ts(t0 // 2, 2 * BT)], et[:])

        NG = NT // GRP  # 4 groups
        WAVEG = 2       # groups per wave
        state = {}
        for g in range(NG):
            state[g] = dict(
                h=[hpool.tile([128, 2, 2, BT], BF16, tag="h", name=f"h{g}_{p}") for p in range(2)],
                mlp=[mlpool.tile([40, BT], BF16, tag="mlp", name=f"mlp{g}_{p}") for p in range(2)],
                lsp=[mlpool.tile([D, 2, BT], BF16, tag="lsp", name=f"lsp{g}_{p}") for p in range(2)],
            )

        for wv in range(NG // WAVEG):
            groups = list(range(wv * WAVEG, (wv + 1) * WAVEG))
            if wv == 0:
                for g in groups:
                    trunk_group(g, state[g]["h"])
            for i in range(D):
                for g in groups:
                    st_ = state[g]
                    step_group(i, g, st_["h"], st_["mlp"], st_["lsp"])
                # emit next wave's trunk early so the PE has work across
                # the wave boundary
                if i == D - 2 and wv + 1 < NG // WAVEG:
                    for g2 in range((wv + 1) * WAVEG, (wv + 2) * WAVEG):
                        trunk_group(g2, state[g2]["h"])
                if i == D - 1:
                    for g in groups:
                        for p in range(2):
                            epilogue_pair(g, p, state[g]["mlp"][p], state[g]["lsp"][p])

    nc.compile()
    return nc


def _get_nc():
    if "nc" not in _NC_CACHE:
        _NC_CACHE["nc"] = _build_bass()
    return _NC_CACHE["nc"]


def kernel(**inputs):
    import ml_dtypes

    bf16 = ml_dtypes.bfloat16
    inp = {k: np.ascontiguousarray(np.asarray(v, dtype=np.float32)) for k, v in inputs.items()}
    x = inp["inputs"]
    eps = inp["eps"]
    W_in, b_in = inp["W_in"], inp["b_in"]
    W_h, b_h = inp["W_h"], inp["b_h"]
    W_out, b_out = inp["W_out"], inp["b_out"]

    def cb(a):
        return np.ascontiguousarray(a.astype(bf16))

    c = np.ascontiguousarray

    # wx2: [64, D, 2, 128] with correction rows at partition bands 0 and 32
    wx2 = np.zeros((64, D, 2, 128), np.float32)
    ext = W_in[:, HID:, :]  # [D, 7, 256]
    for s in range(2):
        for j in range(D - 1):
            for m in range(2):
                wx2[32 * s + j, :, m, :] = ext[:, j, 128 * m : 128 * (m + 1)]

    bo_band = np.zeros((128, D), np.float32)
    for j in range(4):
        for ch in range(2):
            bo_band[32 * j + ch, :] = b_out[:, ch]

    shared = {
        "w0": cb(inp["sW0"]),
        "w1": cb(inp["sW1"].reshape(2, 128, HID).transpose(1, 0, 2).reshape(128, -1)),
        "w2": cb(inp["sW2"].reshape(2, 128, HID).transpose(1, 0, 2).reshape(128, -1)),
        "wi": cb(W_in[:, :HID, :].reshape(D, 2, 128, HID).transpose(2, 0, 1, 3).reshape(128, -1)),
        "wx2": cb(wx2.reshape(64, -1)),
        "wh": cb(W_h.reshape(D, 2, 128, HID).transpose(2, 0, 1, 3).reshape(128, -1)),
        "wo": cb(W_out.reshape(D, 2, 128, 2).transpose(2, 0, 1, 3).reshape(128, -1)),
        "b0": c(inp["sb0"].reshape(2, 128).T),
        "b1": c(inp["sb1"].reshape(2, 128).T),
        "b2": c(inp["sb2"].reshape(2, 128).T),
        "bi": c(b_in.reshape(D, 2, 128).transpose(2, 0, 1).reshape(128, -1)),
        "bh": c(b_h.reshape(D, 2, 128).transpose(2, 0, 1).reshape(128, -1)),
        "bo": c(bo_band),
    }

    in_maps = []
    for core in range(NCORES):
        sl = slice(core * BC, (core + 1) * BC)
        m = dict(shared)
        m["xT"] = cb(x[sl].T)
        m["epsT"] = c(eps[sl].T)
        in_maps.append(m)

    nc = _get_nc()
    kw = {}
    if TRACE:
        import shutil

        shutil.rmtree("/tmp/ktrace", ignore_errors=True)
        os.makedirs("/tmp/ktrace", exist_ok=True)
        kw = dict(trace=True, trace_cores=[0], tmpdir="/tmp/ktrace")
    res = run_bass_kernel_spmd(nc, in_maps, list(range(NCORES)), **kw)
    if TRACE:
        print(f"HW exec time: {res.exec_time_ns} ns")

    out_mean = np.concatenate([res.results[i]["omT"].T for i in range(NCORES)], axis=0)
    out_sample = np.concatenate([res.results[i]["osT"].T for i in range(NCORES)], axis=0)
    out_logp = np.concatenate([res.results[i]["olT"].T for i in range(NCORES)], axis=0)
    return out_mean, out_sample, out_logp


# revision 17
# speedup vs baseline: 1.3340x; 1.0319x over previous
"""Trainium2 Bass kernel for the autoregressive policy head (nn_ADM_6511170421537).

Structure (per core, pure data parallelism over 8 cores):
  trunk:  h = relu(x@sW0+b) -> relu(@sW1+b) -> relu(@sW2+b)          [B,256]
  steps i=0..7 (sequential in i, batch-parallel):
      x1 = relu(h@W_in[i][:256] + means[:i]@W_in[i][256:256+i] + b_in[i])
      x2 = relu(x1@W_h[i] + b_h[i])
      (mean_i, ls_i) = relu(x2@W_out[i] + b_out[i])
  epilogue (batched over the 8 steps, fp32):
      log_std = min(ls, 2);  std = exp(log_std)
      sample  = mean + std*eps
      logp    = -0.5*eps^2 - log_std - 0.5*log(2pi)   (== reference algebra)

Layout: feature-major on chip ([features->partitions, batch->free]); the host
transposes inputs/eps/outputs so every DMA moves contiguous lines.  Matmuls
run in bf16 (PSUM accumulates fp32), epilogue math in fp32.

Perf structure: batch tiles are processed in GROUPS of 4 (two PAIRS).  All
matmuls sharing a stationary operand are emitted back-to-back (weight-load
hides in the streaming of the previous matmul), a pair shares one 2-bank
PSUM tile so each PSUM->SBUF evacuation covers 2 tiles in one op (FD=1024),
the tiny M=2 head matmuls of the 4 tiles in a group run CONCURRENTLY in
disjoint PE column groups, and the small K=i "autoregressive correction"
matmuls of a pair run concurrently in disjoint PE row groups (the means are
DMA-scattered into per-slot 32-partition bands).
"""

import os

os.environ.setdefault("MYCRO_LOCAL_CACHE", "1")

import numpy as np
from contextlib import ExitStack

import concourse.bass as bass
import concourse.bacc as bacc
import concourse.mybir as mybir
import concourse.tile as tile
from concourse.bass_utils import run_bass_kernel_spmd

# ---- problem constants (hardcoded; kernel.py must be self-contained) ----
B = 65536
IN_DIM = 64
HID = 256
D = 8
NCORES = 8
BC = B // NCORES          # 8192 rows per core
BT = 512                  # batch tile (one fp32 PSUM bank of free dim)
NT = BC // BT             # 16 tiles per core
GRP = 4                   # tiles per group (head col-tiling width)
LOG_2PI = float(np.log(2.0 * np.pi))

F32 = mybir.dt.float32
BF16 = mybir.dt.bfloat16
RELU = mybir.ActivationFunctionType.Relu
EXP = mybir.ActivationFunctionType.Exp
ADD = mybir.AluOpType.add
MAX = mybir.AluOpType.max
MIN = mybir.AluOpType.min
MULT = mybir.AluOpType.mult

TRACE = False           # test.py flips this to get the NTFF profile
_NC_CACHE = {}


def _build_bass():
    nc = bacc.Bacc()

    xT = nc.declare_dram_parameter("xT", [IN_DIM, BC], BF16, isOutput=False)
    epsT = nc.declare_dram_parameter("epsT", [D, BC], F32, isOutput=False)
    w0 = nc.declare_dram_parameter("w0", [IN_DIM, HID], BF16, isOutput=False)
    w1 = nc.declare_dram_parameter("w1", [128, 2 * HID], BF16, isOutput=False)
    w2 = nc.declare_dram_parameter("w2", [128, 2 * HID], BF16, isOutput=False)
    wi = nc.declare_dram_parameter("wi", [128, D * 2 * HID], BF16, isOutput=False)
    # wx2: correction rows replicated in partition bands 0-6 and 32-38 so a
    # pair's two correction matmuls can run in disjoint PE row groups.
    wx2 = nc.declare_dram_parameter("wx2", [64, D * 2 * 128], BF16, isOutput=False)
    wh = nc.declare_dram_parameter("wh", [128, D * 2 * HID], BF16, isOutput=False)
    wo = nc.declare_dram_parameter("wo", [128, D * 2 * 2], BF16, isOutput=False)
    b0 = nc.declare_dram_parameter("b0", [128, 2], F32, isOutput=False)
    b1 = nc.declare_dram_parameter("b1", [128, 2], F32, isOutput=False)
    b2 = nc.declare_dram_parameter("b2", [128, 2], F32, isOutput=False)
    bi = nc.declare_dram_parameter("bi", [128, D * 2], F32, isOutput=False)
    bh = nc.declare_dram_parameter("bh", [128, D * 2], F32, isOutput=False)
    bo = nc.declare_dram_parameter("bo", [128, D], F32, isOutput=False)
    omT = nc.declare_dram_parameter("omT", [D, BC], F32, isOutput=True)
    osT = nc.declare_dram_parameter("osT", [D, BC], F32, isOutput=True)
    olT = nc.declare_dram_parameter("olT", [D, BC], F32, isOutput=True)

    with tile.TileContext(nc) as tc, ExitStack() as ctx:
        wp = ctx.enter_context(tc.tile_pool(name="w", bufs=1))
        hpool = ctx.enter_context(tc.tile_pool(name="h", bufs=NT // 2 + 1))
        mlpool = ctx.enter_context(tc.tile_pool(name="ml", bufs=NT + 2))
        xpool = ctx.enter_context(tc.tile_pool(name="xin", bufs=4))
        xtpool = ctx.enter_context(tc.tile_pool(name="xtp", bufs=NT))
        tpool = ctx.enter_context(tc.tile_pool(name="tr", bufs=2))
        epool = ctx.enter_context(tc.tile_pool(name="ep", bufs=2))
        opool = ctx.enter_context(tc.tile_pool(name="out", bufs=1))
        pspair = ctx.enter_context(tc.tile_pool(name="pspair", bufs=4, space="PSUM"))

        # ---- resident weights ----
        w0_s = wp.tile([IN_DIM, HID], BF16)
        nc.sync.dma_start(w0_s[:], w0[:])
        w1_s = wp.tile([128, 2, HID], BF16)
        nc.sync.dma_start(w1_s[:], w1[:].rearrange("p (k m) -> p k m", k=2))
        w2_s = wp.tile([128, 2, HID], BF16)
        nc.sync.dma_start(w2_s[:], w2[:].rearrange("p (k m) -> p k m", k=2))
        wi_s = wp.tile([128, D, 2, HID], BF16)
        nc.sync.dma_start(wi_s[:], wi[:].rearrange("p (i k m) -> p i k m", i=D, k=2))
        wx2_s = wp.tile([64, D, 2, 128], BF16)
        nc.sync.dma_start(wx2_s[:], wx2[:].rearrange("j (i m c) -> j i m c", i=D, m=2))
        wh_s = wp.tile([128, D, 2, HID], BF16)
        nc.sync.dma_start(wh_s[:], wh[:].rearrange("p (i k m) -> p i k m", i=D, k=2))
        wo_s = wp.tile([128, D, 2, 2], BF16)
        nc.sync.dma_start(wo_s[:], wo[:].rearrange("p (i k c) -> p i k c", i=D, k=2))
        b0_s = wp.tile([128, 2], F32)
        nc.sync.dma_start(b0_s[:], b0[:])
        b1_s = wp.tile([128, 2], F32)
        nc.sync.dma_start(b1_s[:], b1[:])
        b2_s = wp.tile([128, 2], F32)
        nc.sync.dma_start(b2_s[:], b2[:])
        bi_s = wp.tile([128, D, 2], F32)
        nc.sync.dma_start(bi_s[:], bi[:].rearrange("p (i m) -> p i m", i=D))
        bh_s = wp.tile([128, D, 2], F32)
        nc.sync.dma_start(bh_s[:], bh[:].rearrange("p (i m) -> p i m", i=D))
        bo_s = wp.tile([128, D], F32)
        nc.sync.dma_start(bo_s[:], bo[:])

        # prefetch ALL input tiles up front (tiny: 16 x 1KB/partition-line)
        xts = []
        for t in range(NT):
            xt = xtpool.tile([IN_DIM, BT], BF16, tag="xt", name=f"xt{t}")
            nc.sync.dma_start(xt[:], xT[:, bass.ts(t, BT)])
            xts.append(xt)

        def evac_act(dst, src, bias):
            nc.scalar.activation(dst, src, RELU, bias=bias)

        def evac_dve(dst, src, bias):
            nc.vector.tensor_scalar(dst, src, bias, 0.0, ADD, MAX)

        # A "pair tile" holds two batch tiles: SBUF [128, m(2), slot(2), BT];
        # PSUM pair tiles are [128, slot(2), BT] (2 banks).

        def layer_pairs(weight_col, bias_col, rhs_of, dst_of, evacs, n_k, corr_i=0,
                        mlps=None):
            """One dense layer over a group of 2 pairs (4 tiles)."""
            for m in range(2):
                pss = [
                    pspair.tile([128, 2, BT], F32, tag="pspair", name=f"ps{m}{p}")
                    for p in range(2)
                ]
                for k in range(n_k):
                    wv = weight_col(k, m)
                    for p in range(2):
                        for s in range(2):
                            nc.tensor.matmul(
                                pss[p][:, s, :], wv, rhs_of(2 * p + s, k),
                                start=(k == 0), stop=(k == n_k - 1 and corr_i == 0),
                            )
                if corr_i > 0:
                    i = corr_i
                    for p in range(2):
                        for s in range(2):
                            # slot s reads its means band at partitions 32s;
                            # the two slots run in disjoint PE row groups.
                            nc.tensor.matmul(
                                pss[p][:, s, :],
                                wx2_s[32 * s : 32 * s + i, i, m, :],
                                mlps[p][32 * s : 32 * s + i, :],
                                start=False, stop=True,
                                tile_position=(32 * s, 0),
                            )
                for p in range(2):
                    evacs[p](dst_of(p, m), pss[p][:], bias_col(m))

        def trunk_group(g, hps):
            """Trunk for tiles [4g..4g+4); writes h into hps[p] pair tiles."""
            t0 = GRP * g
            hp = [tpool.tile([128, 2, 2, BT], BF16, tag=f"hp{p}", name=f"hp{g}_{p}", bufs=1) for p in range(2)]
            layer_pairs(
                lambda k, m: w0_s[:, bass.ts(m, 128)],
                lambda m: b0_s[:, m : m + 1],
                lambda j, k: xts[t0 + j][:],
                lambda p, m: hp[p][:, m, :, :],
                [evac_act, evac_dve],
                n_k=1,
            )
            hq = [tpool.tile([128, 2, 2, BT], BF16, tag=f"hq{p}", name=f"hq{g}_{p}", bufs=1) for p in range(2)]
            layer_pairs(
                lambda k, m: w1_s[:, k, bass.ts(m, 128)],
                lambda m: b1_s[:, m : m + 1],
                lambda j, k: hp[j // 2][:, k, j % 2, :],
                lambda p, m: hq[p][:, m, :, :],
                [evac_dve, evac_act],
                n_k=2,
            )
            layer_pairs(
                lambda k, m: w2_s[:, k, bass.ts(m, 128)],
                lambda m: b2_s[:, m : m + 1],
                lambda j, k: hq[j // 2][:, k, j % 2, :],
                lambda p, m: hps[p][:, m, :, :],
                [evac_act, evac_dve],
                n_k=2,
            )

        def step_mlps(i, g, hps, mlps):
            """Step i MLP part (L_in + L_h) for the 4 tiles of group g."""
            x1 = [tpool.tile([128, 2, 2, BT], BF16, tag=f"x1{p}", name=f"x1{g}_{p}") for p in range(2)]
            layer_pairs(
                lambda k, m: wi_s[:, i, k, bass.ts(m, 128)],
                lambda m: bi_s[:, i, m : m + 1],
                lambda j, k: hps[j // 2][:, k, j % 2, :],
                lambda p, m: x1[p][:, m, :, :],
                [evac_act, evac_dve],
                n_k=2,
                corr_i=i,
                mlps=mlps,
            )
            x2 = [tpool.tile([128, 2, 2, BT], BF16, tag=f"x2{p}", name=f"x2{g}_{p}") for p in range(2)]
            layer_pairs(
                lambda k, m: wh_s[:, i, k, bass.ts(m, 128)],
                lambda m: bh_s[:, i, m : m + 1],
                lambda j, k: x1[j // 2][:, k, j % 2, :],
                lambda p, m: x2[p][:, m, :, :],
                [evac_dve, evac_act],
                n_k=2,
            )
            return x2

        def step_head(i, g, x2, mlps, lsps):
            # head: the 4 tiles' M=2 matmuls run concurrently in distinct
            # PE column groups (tile_position), landing at psum partitions 32j.
            pst = pspair.tile([128, 2, BT], F32, tag="pspair", name=f"psh{g}")
            pso = pst[:, 0, :]
            for k in range(2):
                for j in range(GRP):
                    nc.tensor.matmul(
                        pso[32 * j : 32 * j + 2, :],
                        wo_s[:, i, k, :],
                        x2[j // 2][:, k, j % 2, :],
                        start=(k == 0), stop=(k == 1),
                        tile_position=(0, 32 * j),
                    )
            sm = xpool.tile([128, BT], BF16, tag="sm", name=f"sm{g}_{i}")
            evac_act(sm[0:98, :], pso[0:98, :], bo_s[0:98, i : i + 1])
            for j in range(GRP):
                p, s = j // 2, j % 2
                nc.sync.dma_start(
                    mlps[p][32 * s + i : 32 * s + i + 1, :], sm[32 * j : 32 * j + 1, :]
                )
                nc.sync.dma_start(
                    lsps[p][i : i + 1, s, :], sm[32 * j + 1 : 32 * j + 2, :]
                )

        def epilogue_pair(g, p, mlp, lsp):
            """Epilogue for pair p of group g (two tiles, FD=1024 ops)."""
            t0 = GRP * g + 2 * p
            et = epool.tile([D, 2, BT], F32, tag="et")
            nc.sync.dma_start(et[:], epsT[:, bass.ts(t0 // 2, 2 * BT)])
            mean_f = opool.tile([D, 2, BT], F32, tag="mean_f")
            nc.vector.tensor_copy(mean_f[:, 0, :], mlp[0:D, :])
            nc.vector.tensor_copy(mean_f[:, 1, :], mlp[32 : 32 + D, :])
            ls = opool.tile([D, 2, BT], F32, tag="ls")
            nc.vector.tensor_single_scalar(ls[:], lsp[:], 2.0, MIN)
            st = opool.tile([D, 2, BT], F32, tag="st")
            nc.scalar.activation(st[:], ls[:], EXP)
            nc.sync.dma_start(omT[:, bass.ts(t0 // 2, 2 * BT)], mean_f[:])
            # elementwise in-place: DVE writes trail reads through the pipe
            nc.vector.tensor_mul(st[:], st[:], et[:])            # st*eps
            nc.vector.tensor_add(st[:], st[:], mean_f[:])        # sample
            nc.sync.dma_start(osT[:, bass.ts(t0 // 2, 2 * BT)], st[:])
            nc.vector.tensor_mul(et[:], et[:], et[:])            # eps^2
            nc.vector.tensor_scalar(et[:], et[:], -0.5, -0.5 * LOG_2PI, MULT, ADD)
            nc.vector.tensor_sub(et[:], et[:], ls[:])            # logp
            nc.sync.dma_start(olT[:, bass.ts(t0 // 2, 2 * BT)], et[:])

        NG = NT // GRP  # 4 groups
        WAVEG = 2       # groups per wave
        state = {}
        for g in range(NG):
            state[g] = dict(
                h=[hpool.tile([128, 2, 2, BT], BF16, tag="h", name=f"h{g}_{p}") for p in range(2)],
                mlp=[mlpool.tile([40, BT], BF16, tag="mlp", name=f"mlp{g}_{p}") for p in range(2)],
                lsp=[mlpool.tile([D, 2, BT], BF16, tag="lsp", name=f"lsp{g}_{p}") for p in range(2)],
            )

        for wv in range(NG // WAVEG):
            groups = list(range(wv * WAVEG, (wv + 1) * WAVEG))
            if wv == 0:
                for g in groups:
                    trunk_group(g, state[g]["h"])
            for i in range(D):
                x2s = {}
                for g in groups:
                    st_ = state[g]
                    x2s[g] = step_mlps(i, g, st_["h"], st_["mlp"])
                for g in groups:
                    st_ = state[g]
                    step_head(i, g, x2s[g], st_["mlp"], st_["lsp"])
                # emit next wave's trunk early so the PE has work across
                # the wave boundary
                if i == D - 2 and wv + 1 < NG // WAVEG:
                    for g2 in range((wv + 1) * WAVEG, (wv + 2) * WAVEG):
                        trunk_group(g2, state[g2]["h"])
                if i == D - 1:
                    for g in groups:
                        for p in range(2):
                            epilogue_pair(g, p, state[g]["mlp"][p], state[g]["lsp"][p])

    nc.compile()
    return nc


def _get_nc():
    if "nc" not in _NC_CACHE:
        _NC_CACHE["nc"] = _build_bass()
    return _NC_CACHE["nc"]


def kernel(**inputs):
    import ml_dtypes

    bf16 = ml_dtypes.bfloat16
    inp = {k: np.ascontiguousarray(np.asarray(v, dtype=np.float32)) for k, v in inputs.items()}
    x = inp["inputs"]
    eps = inp["eps"]
    W_in, b_in = inp["W_in"], inp["b_in"]
    W_h, b_h = inp["W_h"], inp["b_h"]
    W_out, b_out = inp["W_out"], inp["b_out"]

    def cb(a):
        return np.ascontiguousarray(a.astype(bf16))

    c = np.ascontiguousarray

    # wx2: [64, D, 2, 128] with correction rows at partition bands 0 and 32
    wx2 = np.zeros((64, D, 2, 128), np.float32)
    ext = W_in[:, HID:, :]  # [D, 7, 256]
    for s in range(2):
        for j in range(D - 1):
            for m in range(2):
                wx2[32 * s + j, :, m, :] = ext[:, j, 128 * m : 128 * (m + 1)]

    bo_band = np.zeros((128, D), np.float32)
    for j in range(4):
        for ch in range(2):
            bo_band[32 * j + ch, :] = b_out[:, ch]

    shared = {
        "w0": cb(inp["sW0"]),
        "w1": cb(inp["sW1"].reshape(2, 128, HID).transpose(1, 0, 2).reshape(128, -1)),
        "w2": cb(inp["sW2"].reshape(2, 128, HID).transpose(1, 0, 2).reshape(128, -1)),
        "wi": cb(W_in[:, :HID, :].reshape(D, 2, 128, HID).transpose(2, 0, 1, 3).reshape(128, -1)),
        "wx2": cb(wx2.reshape(64, -1)),
        "wh": cb(W_h.reshape(D, 2, 128, HID).transpose(2, 0, 1, 3).reshape(128, -1)),
        "wo": cb(W_out.reshape(D, 2, 128, 2).transpose(2, 0, 1, 3).reshape(128, -1)),
        "b0": c(inp["sb0"].reshape(2, 128).T),
        "b1": c(inp["sb1"].reshape(2, 128).T),
        "b2": c(inp["sb2"].reshape(2, 128).T),
        "bi": c(b_in.reshape(D, 2, 128).transpose(2, 0, 1).reshape(128, -1)),
        "bh": c(b_h.reshape(D, 2, 128).transpose(2, 0, 1).reshape(128, -1)),
        "bo": c(bo_band),
    }

    in_maps = []
    for core in range(NCORES):
        sl = slice(core * BC, (core + 1) * BC)
        m = dict(shared)
        m["xT"] = cb(x[sl].T)
        m["epsT"] = c(eps[sl].T)
        in_maps.append(m)

    nc = _get_nc()
    kw = {}
    if TRACE:
        import shutil

        shutil.rmtree("/tmp/ktrace", ignore_errors=True)
        os.makedirs("/tmp/ktrace", exist_ok=True)
        kw = dict(trace=True, trace_cores=[0], tmpdir="/tmp/ktrace")
    res = run_bass_kernel_spmd(nc, in_maps, list(range(NCORES)), **kw)
    if TRACE:
        print(f"HW exec time: {res.exec_time_ns} ns")

    out_mean = np.concatenate([res.results[i]["omT"].T for i in range(NCORES)], axis=0)
    out_sample = np.concatenate([res.results[i]["osT"].T for i in range(NCORES)], axis=0)
    out_logp = np.concatenate([res.results[i]["olT"].T for i in range(NCORES)], axis=0)
    return out_mean, out_sample, out_logp


# revision 18
# speedup vs baseline: 1.3567x; 1.0170x over previous
"""Trainium2 Bass kernel for the autoregressive policy head (nn_ADM_6511170421537).

Structure (per core, pure data parallelism over 8 cores):
  trunk:  h = relu(x@sW0+b) -> relu(@sW1+b) -> relu(@sW2+b)          [B,256]
  steps i=0..7 (sequential in i, batch-parallel):
      x1 = relu(h@W_in[i][:256] + means[:i]@W_in[i][256:256+i] + b_in[i])
      x2 = relu(x1@W_h[i] + b_h[i])
      (mean_i, ls_i) = relu(x2@W_out[i] + b_out[i])
  epilogue (batched over the 8 steps, fp32):
      log_std = min(ls, 2);  std = exp(log_std)
      sample  = mean + std*eps
      logp    = -0.5*eps^2 - log_std - 0.5*log(2pi)   (== reference algebra)

Layout: feature-major on chip ([features->partitions, batch->free]); the host
transposes inputs/eps/outputs so every DMA moves contiguous lines.  Matmuls
run in bf16 (PSUM accumulates fp32), epilogue math in fp32.

Perf structure: batch tiles are processed in GROUPS of 4 (two PAIRS).  All
matmuls sharing a stationary operand are emitted back-to-back (weight-load
hides in the streaming of the previous matmul), a pair shares one 2-bank
PSUM tile so each PSUM->SBUF evacuation covers 2 tiles in one op (FD=1024),
the tiny M=2 head matmuls of the 4 tiles in a group run CONCURRENTLY in
disjoint PE column groups, and the small K=i "autoregressive correction"
matmuls of a pair run concurrently in disjoint PE row groups (the means are
DMA-scattered into per-slot 32-partition bands).
"""

import os

os.environ.setdefault("MYCRO_LOCAL_CACHE", "1")

import numpy as np
from contextlib import ExitStack

import concourse.bass as bass
import concourse.bacc as bacc
import concourse.mybir as mybir
import concourse.tile as tile
from concourse.bass_utils import run_bass_kernel_spmd

# ---- problem constants (hardcoded; kernel.py must be self-contained) ----
B = 65536
IN_DIM = 64
HID = 256
D = 8
NCORES = 8
BC = B // NCORES          # 8192 rows per core
BT = 512                  # batch tile (one fp32 PSUM bank of free dim)
NT = BC // BT             # 16 tiles per core
GRP = 4                   # tiles per group (head col-tiling width)
LOG_2PI = float(np.log(2.0 * np.pi))

F32 = mybir.dt.float32
BF16 = mybir.dt.bfloat16
RELU = mybir.ActivationFunctionType.Relu
EXP = mybir.ActivationFunctionType.Exp
ADD = mybir.AluOpType.add
MAX = mybir.AluOpType.max
MIN = mybir.AluOpType.min
MULT = mybir.AluOpType.mult

TRACE = False           # test.py flips this to get the NTFF profile
_NC_CACHE = {}


def _build_bass():
    nc = bacc.Bacc()

    xT = nc.declare_dram_parameter("xT", [IN_DIM, BC], BF16, isOutput=False)
    epsT = nc.declare_dram_parameter("epsT", [D, BC], BF16, isOutput=False)
    w0 = nc.declare_dram_parameter("w0", [IN_DIM, HID], BF16, isOutput=False)
    w1 = nc.declare_dram_parameter("w1", [128, 2 * HID], BF16, isOutput=False)
    w2 = nc.declare_dram_parameter("w2", [128, 2 * HID], BF16, isOutput=False)
    wi = nc.declare_dram_parameter("wi", [128, D * 2 * HID], BF16, isOutput=False)
    # wx2: correction rows replicated in partition bands 0-6 and 32-38 so a
    # pair's two correction matmuls can run in disjoint PE row groups.
    wx2 = nc.declare_dram_parameter("wx2", [64, D * 2 * 128], BF16, isOutput=False)
    wh = nc.declare_dram_parameter("wh", [128, D * 2 * HID], BF16, isOutput=False)
    wo = nc.declare_dram_parameter("wo", [128, D * 2 * 2], BF16, isOutput=False)
    b0 = nc.declare_dram_parameter("b0", [128, 2], F32, isOutput=False)
    b1 = nc.declare_dram_parameter("b1", [128, 2], F32, isOutput=False)
    b2 = nc.declare_dram_parameter("b2", [128, 2], F32, isOutput=False)
    bi = nc.declare_dram_parameter("bi", [128, D * 2], F32, isOutput=False)
    bh = nc.declare_dram_parameter("bh", [128, D * 2], F32, isOutput=False)
    bo = nc.declare_dram_parameter("bo", [128, D], F32, isOutput=False)
    omT = nc.declare_dram_parameter("omT", [D, BC], F32, isOutput=True)
    osT = nc.declare_dram_parameter("osT", [D, BC], F32, isOutput=True)
    olT = nc.declare_dram_parameter("olT", [D, BC], F32, isOutput=True)

    with tile.TileContext(nc) as tc, ExitStack() as ctx:
        wp = ctx.enter_context(tc.tile_pool(name="w", bufs=1))
        hpool = ctx.enter_context(tc.tile_pool(name="h", bufs=NT // 2 + 1))
        mlpool = ctx.enter_context(tc.tile_pool(name="ml", bufs=NT + 2))
        xpool = ctx.enter_context(tc.tile_pool(name="xin", bufs=4))
        xtpool = ctx.enter_context(tc.tile_pool(name="xtp", bufs=NT))
        tpool = ctx.enter_context(tc.tile_pool(name="tr", bufs=2))
        epool = ctx.enter_context(tc.tile_pool(name="ep", bufs=2))
        opool = ctx.enter_context(tc.tile_pool(name="out", bufs=1))
        pspair = ctx.enter_context(tc.tile_pool(name="pspair", bufs=4, space="PSUM"))

        # ---- inputs + trunk weights first (unblock the first matmuls),
        # ---- the two 1MB step-weight blobs last (stream under the trunk)
        xts = []
        for t in range(NT):
            xt = xtpool.tile([IN_DIM, BT], BF16, tag="xt", name=f"xt{t}")
            nc.sync.dma_start(xt[:], xT[:, bass.ts(t, BT)])
            xts.append(xt)
        w0_s = wp.tile([IN_DIM, HID], BF16)
        nc.sync.dma_start(w0_s[:], w0[:])
        b0_s = wp.tile([128, 2], F32)
        nc.sync.dma_start(b0_s[:], b0[:])
        w1_s = wp.tile([128, 2, HID], BF16)
        nc.sync.dma_start(w1_s[:], w1[:].rearrange("p (k m) -> p k m", k=2))
        b1_s = wp.tile([128, 2], F32)
        nc.sync.dma_start(b1_s[:], b1[:])
        w2_s = wp.tile([128, 2, HID], BF16)
        nc.sync.dma_start(w2_s[:], w2[:].rearrange("p (k m) -> p k m", k=2))
        b2_s = wp.tile([128, 2], F32)
        nc.sync.dma_start(b2_s[:], b2[:])
        wx2_s = wp.tile([64, D, 2, 128], BF16)
        nc.sync.dma_start(wx2_s[:], wx2[:].rearrange("j (i m c) -> j i m c", i=D, m=2))
        wo_s = wp.tile([128, D, 2, 2], BF16)
        nc.sync.dma_start(wo_s[:], wo[:].rearrange("p (i k c) -> p i k c", i=D, k=2))
        bi_s = wp.tile([128, D, 2], F32)
        nc.sync.dma_start(bi_s[:], bi[:].rearrange("p (i m) -> p i m", i=D))
        bh_s = wp.tile([128, D, 2], F32)
        nc.sync.dma_start(bh_s[:], bh[:].rearrange("p (i m) -> p i m", i=D))
        bo_s = wp.tile([128, D], F32)
        nc.sync.dma_start(bo_s[:], bo[:])
        wi_s = wp.tile([128, D, 2, HID], BF16)
        nc.sync.dma_start(wi_s[:], wi[:].rearrange("p (i k m) -> p i k m", i=D, k=2))
        wh_s = wp.tile([128, D, 2, HID], BF16)
        nc.sync.dma_start(wh_s[:], wh[:].rearrange("p (i k m) -> p i k m", i=D, k=2))

        def evac_act(dst, src, bias):
            nc.scalar.activation(dst, src, RELU, bias=bias)

        def evac_dve(dst, src, bias):
            nc.vector.tensor_scalar(dst, src, bias, 0.0, ADD, MAX)

        # A "pair tile" holds two batch tiles: SBUF [128, m(2), slot(2), BT];
        # PSUM pair tiles are [128, slot(2), BT] (2 banks).

        def layer_pairs(weight_col, bias_col, rhs_of, dst_of, evacs, n_k, corr_i=0,
                        mlps=None):
            """One dense layer over a group of 2 pairs (4 tiles)."""
            for m in range(2):
                pss = [
                    pspair.tile([128, 2, BT], F32, tag="pspair", name=f"ps{m}{p}")
                    for p in range(2)
                ]
                for k in range(n_k):
                    wv = weight_col(k, m)
                    for p in range(2):
                        for s in range(2):
                            nc.tensor.matmul(
                                pss[p][:, s, :], wv, rhs_of(2 * p + s, k),
                                start=(k == 0), stop=(k == n_k - 1 and corr_i == 0),
                            )
                if corr_i > 0:
                    i = corr_i
                    for p in range(2):
                        for s in range(2):
                            # slot s reads its means band at partitions 32s;
                            # the two slots run in disjoint PE row groups.
                            nc.tensor.matmul(
                                pss[p][:, s, :],
                                wx2_s[32 * s : 32 * s + i, i, m, :],
                                mlps[p][32 * s : 32 * s + i, :],
                                start=False, stop=True,
                                tile_position=(32 * s, 0),
                            )
                for p in range(2):
                    evacs[p](dst_of(p, m), pss[p][:], bias_col(m))

        def trunk_group(g, hps):
            """Trunk for tiles [4g..4g+4); writes h into hps[p] pair tiles."""
            t0 = GRP * g
            hp = [tpool.tile([128, 2, 2, BT], BF16, tag=f"hp{p}", name=f"hp{g}_{p}", bufs=1) for p in range(2)]
            layer_pairs(
                lambda k, m: w0_s[:, bass.ts(m, 128)],
                lambda m: b0_s[:, m : m + 1],
                lambda j, k: xts[t0 + j][:],
                lambda p, m: hp[p][:, m, :, :],
                [evac_act, evac_dve],
                n_k=1,
            )
            hq = [tpool.tile([128, 2, 2, BT], BF16, tag=f"hq{p}", name=f"hq{g}_{p}", bufs=1) for p in range(2)]
            layer_pairs(
                lambda k, m: w1_s[:, k, bass.ts(m, 128)],
                lambda m: b1_s[:, m : m + 1],
                lambda j, k: hp[j // 2][:, k, j % 2, :],
                lambda p, m: hq[p][:, m, :, :],
                [evac_dve, evac_act],
                n_k=2,
            )
            layer_pairs(
                lambda k, m: w2_s[:, k, bass.ts(m, 128)],
                lambda m: b2_s[:, m : m + 1],
                lambda j, k: hq[j // 2][:, k, j % 2, :],
                lambda p, m: hps[p][:, m, :, :],
                [evac_act, evac_dve],
                n_k=2,
            )

        def step_mlps(i, g, hps, mlps):
            """Step i MLP part (L_in + L_h) for the 4 tiles of group g."""
            x1 = [tpool.tile([128, 2, 2, BT], BF16, tag=f"x1{p}", name=f"x1{g}_{p}") for p in range(2)]
            layer_pairs(
                lambda k, m: wi_s[:, i, k, bass.ts(m, 128)],
                lambda m: bi_s[:, i, m : m + 1],
                lambda j, k: hps[j // 2][:, k, j % 2, :],
                lambda p, m: x1[p][:, m, :, :],
                [evac_act, evac_dve],
                n_k=2,
                corr_i=i,
                mlps=mlps,
            )
            x2 = [tpool.tile([128, 2, 2, BT], BF16, tag=f"x2{p}", name=f"x2{g}_{p}") for p in range(2)]
            layer_pairs(
                lambda k, m: wh_s[:, i, k, bass.ts(m, 128)],
                lambda m: bh_s[:, i, m : m + 1],
                lambda j, k: x1[j // 2][:, k, j % 2, :],
                lambda p, m: x2[p][:, m, :, :],
                [evac_dve, evac_act],
                n_k=2,
            )
            return x2

        def step_head(i, g, x2, mlps, lsps):
            # head: the 4 tiles' M=2 matmuls run concurrently in distinct
            # PE column groups (tile_position), landing at psum partitions 32j.
            pst = pspair.tile([128, 2, BT], F32, tag="pspair", name=f"psh{g}")
            pso = pst[:, 0, :]
            for k in range(2):
                for j in range(GRP):
                    nc.tensor.matmul(
                        pso[32 * j : 32 * j + 2, :],
                        wo_s[:, i, k, :],
                        x2[j // 2][:, k, j % 2, :],
                        start=(k == 0), stop=(k == 1),
                        tile_position=(0, 32 * j),
                    )
            sm = xpool.tile([128, BT], BF16, tag="sm", name=f"sm{g}_{i}")
            evac_act(sm[0:98, :], pso[0:98, :], bo_s[0:98, i : i + 1])
            for j in range(GRP):
                p, s = j // 2, j % 2
                nc.sync.dma_start(
                    mlps[p][32 * s + i : 32 * s + i + 1, :], sm[32 * j : 32 * j + 1, :]
                )
                nc.sync.dma_start(
                    lsps[p][i : i + 1, s, :], sm[32 * j + 1 : 32 * j + 2, :]
                )

        def epilogue_pair(g, p, mlp, lsp):
            """Epilogue for pair p of group g (two tiles, FD=1024 ops)."""
            t0 = GRP * g + 2 * p
            et = epool.tile([D, 2, BT], BF16, tag="et")
            nc.sync.dma_start(et[:], epsT[:, bass.ts(t0 // 2, 2 * BT)])
            mean_f = opool.tile([D, 2, BT], F32, tag="mean_f")
            nc.vector.tensor_copy(mean_f[:, 0, :], mlp[0:D, :])
            nc.vector.tensor_copy(mean_f[:, 1, :], mlp[32 : 32 + D, :])
            ls = opool.tile([D, 2, BT], BF16, tag="ls")
            nc.vector.tensor_single_scalar(ls[:], lsp[:], 2.0, MIN)
            st = opool.tile([D, 2, BT], BF16, tag="st")
            nc.scalar.activation(st[:], ls[:], EXP)
            nc.sync.dma_start(omT[:, bass.ts(t0 // 2, 2 * BT)], mean_f[:])
            # elementwise in-place: DVE writes trail reads through the pipe
            nc.vector.tensor_mul(st[:], st[:], et[:])            # st*eps (bf16 2x)
            smp = opool.tile([D, 2, BT], F32, tag="smp")
            nc.vector.tensor_add(smp[:], st[:], mean_f[:])       # sample -> f32
            nc.sync.dma_start(osT[:, bass.ts(t0 // 2, 2 * BT)], smp[:])
            sq = opool.tile([D, 2, BT], BF16, tag="sq")
            nc.vector.tensor_mul(sq[:], et[:], et[:])            # eps^2 (bf16 2x)
            nc.vector.tensor_scalar(sq[:], sq[:], -0.5, -0.5 * LOG_2PI, MULT, ADD)
            lp = opool.tile([D, 2, BT], F32, tag="lp")
            nc.vector.tensor_sub(lp[:], sq[:], ls[:])            # logp -> f32
            nc.sync.dma_start(olT[:, bass.ts(t0 // 2, 2 * BT)], lp[:])

        NG = NT // GRP  # 4 groups
        WAVEG = 2       # groups per wave
        state = {}
        for g in range(NG):
            state[g] = dict(
                h=[hpool.tile([128, 2, 2, BT], BF16, tag="h", name=f"h{g}_{p}") for p in range(2)],
                mlp=[mlpool.tile([40, BT], BF16, tag="mlp", name=f"mlp{g}_{p}") for p in range(2)],
                lsp=[mlpool.tile([D, 2, BT], BF16, tag="lsp", name=f"lsp{g}_{p}") for p in range(2)],
            )

        for wv in range(NG // WAVEG):
            groups = list(range(wv * WAVEG, (wv + 1) * WAVEG))
            if wv == 0:
                for g in groups:
                    trunk_group(g, state[g]["h"])
            for i in range(D):
                x2s = {}
                for g in groups:
                    st_ = state[g]
                    x2s[g] = step_mlps(i, g, st_["h"], st_["mlp"])
                for g in groups:
                    st_ = state[g]
                    step_head(i, g, x2s[g], st_["mlp"], st_["lsp"])
                # emit next wave's trunk early so the PE has work across
                # the wave boundary
                if i == D - 2 and wv + 1 < NG // WAVEG:
                    for g2 in range((wv + 1) * WAVEG, (wv + 2) * WAVEG):
                        trunk_group(g2, state[g2]["h"])
                if i == D - 1:
                    for g in groups:
                        for p in range(2):
                            epilogue_pair(g, p, state[g]["mlp"][p], state[g]["lsp"][p])

    nc.compile()
    return nc


def _get_nc():
    if "nc" not in _NC_CACHE:
        _NC_CACHE["nc"] = _build_bass()
    return _NC_CACHE["nc"]


def kernel(**inputs):
    import ml_dtypes

    bf16 = ml_dtypes.bfloat16
    inp = {k: np.ascontiguousarray(np.asarray(v, dtype=np.float32)) for k, v in inputs.items()}
    x = inp["inputs"]
    eps = inp["eps"]
    W_in, b_in = inp["W_in"], inp["b_in"]
    W_h, b_h = inp["W_h"], inp["b_h"]
    W_out, b_out = inp["W_out"], inp["b_out"]

    def cb(a):
        return np.ascontiguousarray(a.astype(bf16))

    c = np.ascontiguousarray

    # wx2: [64, D, 2, 128] with correction rows at partition bands 0 and 32
    wx2 = np.zeros((64, D, 2, 128), np.float32)
    ext = W_in[:, HID:, :]  # [D, 7, 256]
    for s in range(2):
        for j in range(D - 1):
            for m in range(2):
                wx2[32 * s + j, :, m, :] = ext[:, j, 128 * m : 128 * (m + 1)]

    bo_band = np.zeros((128, D), np.float32)
    for j in range(4):
        for ch in range(2):
            bo_band[32 * j + ch, :] = b_out[:, ch]

    shared = {
        "w0": cb(inp["sW0"]),
        "w1": cb(inp["sW1"].reshape(2, 128, HID).transpose(1, 0, 2).reshape(128, -1)),
        "w2": cb(inp["sW2"].reshape(2, 128, HID).transpose(1, 0, 2).reshape(128, -1)),
        "wi": cb(W_in[:, :HID, :].reshape(D, 2, 128, HID).transpose(2, 0, 1, 3).reshape(128, -1)),
        "wx2": cb(wx2.reshape(64, -1)),
        "wh": cb(W_h.reshape(D, 2, 128, HID).transpose(2, 0, 1, 3).reshape(128, -1)),
        "wo": cb(W_out.reshape(D, 2, 128, 2).transpose(2, 0, 1, 3).reshape(128, -1)),
        "b0": c(inp["sb0"].reshape(2, 128).T),
        "b1": c(inp["sb1"].reshape(2, 128).T),
        "b2": c(inp["sb2"].reshape(2, 128).T),
        "bi": c(b_in.reshape(D, 2, 128).transpose(2, 0, 1).reshape(128, -1)),
        "bh": c(b_h.reshape(D, 2, 128).transpose(2, 0, 1).reshape(128, -1)),
        "bo": c(bo_band),
    }

    in_maps = []
    for core in range(NCORES):
        sl = slice(core * BC, (core + 1) * BC)
        m = dict(shared)
        m["xT"] = cb(x[sl].T)
        m["epsT"] = cb(eps[sl].T)
        in_maps.append(m)

    nc = _get_nc()
    kw = {}
    if TRACE:
        import shutil

        shutil.rmtree("/tmp/ktrace", ignore_errors=True)
        os.makedirs("/tmp/ktrace", exist_ok=True)
        kw = dict(trace=True, trace_cores=[0], tmpdir="/tmp/ktrace")
    res = run_bass_kernel_spmd(nc, in_maps, list(range(NCORES)), **kw)
    if TRACE:
        print(f"HW exec time: {res.exec_time_ns} ns")

    out_mean = np.concatenate([res.results[i]["omT"].T for i in range(NCORES)], axis=0)
    out_sample = np.concatenate([res.results[i]["osT"].T for i in range(NCORES)], axis=0)
    out_logp = np.concatenate([res.results[i]["olT"].T for i in range(NCORES)], axis=0)
    return out_mean, out_sample, out_logp


# revision 20
# speedup vs baseline: 1.3870x; 1.0223x over previous
"""Trainium2 Bass kernel for the autoregressive policy head (nn_ADM_6511170421537).

Structure (per core, pure data parallelism over 8 cores):
  trunk:  h = relu(x@sW0+b) -> relu(@sW1+b) -> relu(@sW2+b)          [B,256]
  steps i=0..7 (sequential in i, batch-parallel):
      x1 = relu(h@W_in[i][:256] + means[:i]@W_in[i][256:256+i] + b_in[i])
      x2 = relu(x1@W_h[i] + b_h[i])
      (mean_i, ls_i) = relu(x2@W_out[i] + b_out[i])
  epilogue (batched over the 8 steps, fp32):
      log_std = min(ls, 2);  std = exp(log_std)
      sample  = mean + std*eps
      logp    = -0.5*eps^2 - log_std - 0.5*log(2pi)   (== reference algebra)

Layout: feature-major on chip ([features->partitions, batch->free]); the host
transposes inputs/eps/outputs so every DMA moves contiguous lines.  Matmuls
run in bf16 (PSUM accumulates fp32), epilogue math in fp32.

Perf structure: batch tiles are processed in GROUPS of 4 (two PAIRS).  All
matmuls sharing a stationary operand are emitted back-to-back (weight-load
hides in the streaming of the previous matmul), a pair shares one 2-bank
PSUM tile so each PSUM->SBUF evacuation covers 2 tiles in one op (FD=1024),
the tiny M=2 head matmuls of the 4 tiles in a group run CONCURRENTLY in
disjoint PE column groups, and the small K=i "autoregressive correction"
matmuls of a pair run concurrently in disjoint PE row groups (the means are
DMA-scattered into per-slot 32-partition bands).
"""

import os

os.environ.setdefault("MYCRO_LOCAL_CACHE", "1")

import numpy as np
from contextlib import ExitStack

import concourse.bass as bass
import concourse.bacc as bacc
import concourse.mybir as mybir
import concourse.tile as tile
from concourse.bass_utils import run_bass_kernel_spmd

# ---- problem constants (hardcoded; kernel.py must be self-contained) ----
B = 65536
IN_DIM = 64
HID = 256
D = 8
NCORES = 8
BC = B // NCORES          # 8192 rows per core
BT = 512                  # batch tile (one fp32 PSUM bank of free dim)
NT = BC // BT             # 16 tiles per core
GRP = 4                   # tiles per group (head col-tiling width)
LOG_2PI = float(np.log(2.0 * np.pi))

F32 = mybir.dt.float32
BF16 = mybir.dt.bfloat16
RELU = mybir.ActivationFunctionType.Relu
EXP = mybir.ActivationFunctionType.Exp
ADD = mybir.AluOpType.add
MAX = mybir.AluOpType.max
MIN = mybir.AluOpType.min
MULT = mybir.AluOpType.mult

TRACE = False           # test.py flips this to get the NTFF profile
_NC_CACHE = {}


def _build_bass():
    nc = bacc.Bacc()

    xT = nc.declare_dram_parameter("xT", [IN_DIM, BC], BF16, isOutput=False)
    epsT = nc.declare_dram_parameter("epsT", [D, BC], BF16, isOutput=False)
    # wa: trunk + small weights [w0pad | wx2pad | w1 | w2 | wo]; wb: [wi | wh]
    wa = nc.declare_dram_parameter("wa", [128, 3360], BF16, isOutput=False)
    wbig = nc.declare_dram_parameter("wbig", [128, 8192], BF16, isOutput=False)
    bb = nc.declare_dram_parameter("bb", [128, 46], F32, isOutput=False)
    omT = nc.declare_dram_parameter("omT", [D, BC], F32, isOutput=True)
    osT = nc.declare_dram_parameter("osT", [D, BC], F32, isOutput=True)
    olT = nc.declare_dram_parameter("olT", [D, BC], F32, isOutput=True)

    with tile.TileContext(nc) as tc, ExitStack() as ctx:
        wp = ctx.enter_context(tc.tile_pool(name="w", bufs=1))
        hpool = ctx.enter_context(tc.tile_pool(name="h", bufs=NT // 2 + 1))
        mlpool = ctx.enter_context(tc.tile_pool(name="ml", bufs=NT + 2))
        xpool = ctx.enter_context(tc.tile_pool(name="xin", bufs=4))
        xtpool = ctx.enter_context(tc.tile_pool(name="xtp", bufs=NT))
        tpool = ctx.enter_context(tc.tile_pool(name="tr", bufs=2))
        opool = ctx.enter_context(tc.tile_pool(name="out", bufs=1))
        pspair = ctx.enter_context(tc.tile_pool(name="pspair", bufs=4, space="PSUM"))

        # ---- batched loads: inputs + trunk weights first (unblock the
        # ---- first matmuls), the big step-weight blob last
        xts_s = wp.tile([IN_DIM, NT, BT], BF16)
        nc.sync.dma_start(xts_s[:], xT[:].rearrange("p (t n) -> p t n", t=NT))
        xts = [xts_s[:, t, :] for t in range(NT)]
        wa_s = wp.tile([128, 3360], BF16)
        nc.sync.dma_start(wa_s[:], wa[:])
        bb_s = wp.tile([128, 46], F32)
        nc.sync.dma_start(bb_s[:], bb[:])
        eps_s = wp.tile([D, NT, BT], BF16)
        nc.sync.dma_start(eps_s[:], epsT[:].rearrange("p (t n) -> p t n", t=NT))
        wbig_s = wp.tile([128, 8192], BF16)
        nc.sync.dma_start(wbig_s[:], wbig[:])

        w0_s = wa_s[0:IN_DIM, 0:HID]
        wx2_s = wa_s[0:64, 256:2304].rearrange("j (i m c) -> j i m c", i=D, m=2)
        w1_s = wa_s[:, 2304:2816].rearrange("p (k m) -> p k m", k=2)
        w2_s = wa_s[:, 2816:3328].rearrange("p (k m) -> p k m", k=2)
        wo_s = wa_s[:, 3328:3360].rearrange("p (i k c) -> p i k c", i=D, k=2)
        wi_s = wbig_s[:, 0:4096].rearrange("p (i k m) -> p i k m", i=D, k=2)
        wh_s = wbig_s[:, 4096:8192].rearrange("p (i k m) -> p i k m", i=D, k=2)
        b0_s = bb_s[:, 0:2]
        b1_s = bb_s[:, 2:4]
        b2_s = bb_s[:, 4:6]
        bi_s = bb_s[:, 6:22].rearrange("p (i m) -> p i m", i=D)
        bh_s = bb_s[:, 22:38].rearrange("p (i m) -> p i m", i=D)
        bo_s = bb_s[:, 38:46]

        def evac_act(dst, src, bias):
            nc.scalar.activation(dst, src, RELU, bias=bias)

        def evac_dve(dst, src, bias):
            nc.vector.tensor_scalar(dst, src, bias, 0.0, ADD, MAX)

        # A "pair tile" holds two batch tiles: SBUF [128, m(2), slot(2), BT];
        # PSUM pair tiles are [128, slot(2), BT] (2 banks).

        def layer_pairs(weight_col, bias_col, rhs_of, dst_of, evacs, n_k, corr_i=0,
                        mlps=None):
            """One dense layer over a group of 2 pairs (4 tiles)."""
            for m in range(2):
                pss = [
                    pspair.tile([128, 2, BT], F32, tag="pspair", name=f"ps{m}{p}")
                    for p in range(2)
                ]
                for k in range(n_k):
                    wv = weight_col(k, m)
                    for p in range(2):
                        for s in range(2):
                            nc.tensor.matmul(
                                pss[p][:, s, :], wv, rhs_of(2 * p + s, k),
                                start=(k == 0), stop=(k == n_k - 1 and corr_i == 0),
                            )
                if corr_i > 0:
                    i = corr_i
                    for p in range(2):
                        for s in range(2):
                            # slot s reads its means band at partitions 32s;
                            # the two slots run in disjoint PE row groups.
                            nc.tensor.matmul(
                                pss[p][:, s, :],
                                wx2_s[32 * s : 32 * s + i, i, m, :],
                                mlps[p][32 * s : 32 * s + i, :],
                                start=False, stop=True,
                                tile_position=(32 * s, 0),
                            )
                for p in range(2):
                    evacs[p](dst_of(p, m), pss[p][:], bias_col(m))

        def trunk_group(g, hps):
            """Trunk for tiles [4g..4g+4); writes h into hps[p] pair tiles."""
            t0 = GRP * g
            hp = [tpool.tile([128, 2, 2, BT], BF16, tag=f"hp{p}", name=f"hp{g}_{p}", bufs=1) for p in range(2)]
            layer_pairs(
                lambda k, m: w0_s[:, bass.ts(m, 128)],
                lambda m: b0_s[:, m : m + 1],
                lambda j, k: xts[t0 + j],
                lambda p, m: hp[p][:, m, :, :],
                [evac_act, evac_dve],
                n_k=1,
            )
            hq = [tpool.tile([128, 2, 2, BT], BF16, tag=f"hq{p}", name=f"hq{g}_{p}", bufs=1) for p in range(2)]
            layer_pairs(
                lambda k, m: w1_s[:, k, bass.ts(m, 128)],
                lambda m: b1_s[:, m : m + 1],
                lambda j, k: hp[j // 2][:, k, j % 2, :],
                lambda p, m: hq[p][:, m, :, :],
                [evac_dve, evac_act],
                n_k=2,
            )
            layer_pairs(
                lambda k, m: w2_s[:, k, bass.ts(m, 128)],
                lambda m: b2_s[:, m : m + 1],
                lambda j, k: hq[j // 2][:, k, j % 2, :],
                lambda p, m: hps[p][:, m, :, :],
                [evac_act, evac_dve],
                n_k=2,
            )

        def step_mlps(i, g, hps, mlps):
            """Step i MLP part (L_in + L_h) for the 4 tiles of group g."""
            x1 = [tpool.tile([128, 2, 2, BT], BF16, tag=f"x1{p}", name=f"x1{g}_{p}") for p in range(2)]
            layer_pairs(
                lambda k, m: wi_s[:, i, k, bass.ts(m, 128)],
                lambda m: bi_s[:, i, m : m + 1],
                lambda j, k: hps[j // 2][:, k, j % 2, :],
                lambda p, m: x1[p][:, m, :, :],
                [evac_act, evac_dve],
                n_k=2,
                corr_i=i,
                mlps=mlps,
            )
            x2 = [tpool.tile([128, 2, 2, BT], BF16, tag=f"x2{p}", name=f"x2{g}_{p}") for p in range(2)]
            layer_pairs(
                lambda k, m: wh_s[:, i, k, bass.ts(m, 128)],
                lambda m: bh_s[:, i, m : m + 1],
                lambda j, k: x1[j // 2][:, k, j % 2, :],
                lambda p, m: x2[p][:, m, :, :],
                [evac_dve, evac_act],
                n_k=2,
            )
            return x2

        def step_head(i, g, x2, mlps, lsps):
            # head: the 4 tiles' M=2 matmuls run concurrently in distinct
            # PE column groups (tile_position), landing at psum partitions 32j.
            pst = pspair.tile([128, 2, BT], F32, tag="pspair", name=f"psh{g}")
            pso = pst[:, 0, :]
            for k in range(2):
                for j in range(GRP):
                    nc.tensor.matmul(
                        pso[32 * j : 32 * j + 2, :],
                        wo_s[:, i, k, :],
                        x2[j // 2][:, k, j % 2, :],
                        start=(k == 0), stop=(k == 1),
                        tile_position=(0, 32 * j),
                    )
            sm = xpool.tile([128, BT], BF16, tag="sm", name=f"sm{g}_{i}")
            evac_act(sm[0:98, :], pso[0:98, :], bo_s[0:98, i : i + 1])
            for j in range(GRP):
                p, s = j // 2, j % 2
                nc.sync.dma_start(
                    mlps[p][32 * s + i : 32 * s + i + 1, :], sm[32 * j : 32 * j + 1, :]
                )
                nc.gpsimd.dma_start(
                    lsps[p][i : i + 1, s, :], sm[32 * j + 1 : 32 * j + 2, :]
                )

        def epilogue_pair(g, p, mlp, lsp):
            """Epilogue for pair p of group g (two tiles, FD=1024 ops)."""
            t0 = GRP * g + 2 * p
            et = eps_s[:, t0 : t0 + 2, :]
            mean_f = opool.tile([D, 2, BT], F32, tag="mean_f")
            nc.vector.tensor_copy(mean_f[:, 0, :], mlp[0:D, :])
            nc.vector.tensor_copy(mean_f[:, 1, :], mlp[32 : 32 + D, :])
            ls = opool.tile([D, 2, BT], BF16, tag="ls")
            nc.vector.tensor_single_scalar(ls[:], lsp[:], 2.0, MIN)
            st = opool.tile([D, 2, BT], BF16, tag="st")
            nc.scalar.activation(st[:], ls[:], EXP)
            nc.sync.dma_start(omT[:, bass.ts(t0 // 2, 2 * BT)], mean_f[:])
            # elementwise in-place: DVE writes trail reads through the pipe.
            # mean_f carries mean -> sample -> logp between the output DMAs.
            nc.vector.tensor_mul(st[:], st[:], et)               # std*eps (bf16 2x)
            nc.vector.tensor_add(mean_f[:], st[:], mean_f[:])    # sample -> f32
            nc.sync.dma_start(osT[:, bass.ts(t0 // 2, 2 * BT)], mean_f[:])
            sq = opool.tile([D, 2, BT], BF16, tag="sq")
            nc.vector.tensor_mul(sq[:], et, et)                  # eps^2 (bf16 2x)
            nc.vector.tensor_scalar(sq[:], sq[:], -0.5, -0.5 * LOG_2PI, MULT, ADD)
            nc.vector.tensor_sub(mean_f[:], sq[:], ls[:])        # logp -> f32
            nc.sync.dma_start(olT[:, bass.ts(t0 // 2, 2 * BT)], mean_f[:])

        NG = NT // GRP  # 4 groups
        WAVEG = 2       # groups per wave
        state = {}
        for g in range(NG):
            state[g] = dict(
                h=[hpool.tile([128, 2, 2, BT], BF16, tag="h", name=f"h{g}_{p}") for p in range(2)],
                mlp=[mlpool.tile([40, BT], BF16, tag="mlp", name=f"mlp{g}_{p}") for p in range(2)],
                lsp=[mlpool.tile([D, 2, BT], BF16, tag="lsp", name=f"lsp{g}_{p}") for p in range(2)],
            )

        for wv in range(NG // WAVEG):
            groups = list(range(wv * WAVEG, (wv + 1) * WAVEG))
            if wv == 0:
                for g in groups:
                    trunk_group(g, state[g]["h"])
            for i in range(D):
                x2s = {}
                for g in groups:
                    st_ = state[g]
                    x2s[g] = step_mlps(i, g, st_["h"], st_["mlp"])
                for g in groups:
                    st_ = state[g]
                    step_head(i, g, x2s[g], st_["mlp"], st_["lsp"])
                # emit next wave's trunk early so the PE has work across
                # the wave boundary
                if i == D - 2 and wv + 1 < NG // WAVEG:
                    for g2 in range((wv + 1) * WAVEG, (wv + 2) * WAVEG):
                        trunk_group(g2, state[g2]["h"])
                if i == D - 1:
                    for g in groups:
                        for p in range(2):
                            epilogue_pair(g, p, state[g]["mlp"][p], state[g]["lsp"][p])

    nc.compile()
    return nc


def _get_nc():
    if "nc" not in _NC_CACHE:
        _NC_CACHE["nc"] = _build_bass()
    return _NC_CACHE["nc"]


def kernel(**inputs):
    import ml_dtypes

    bf16 = ml_dtypes.bfloat16
    inp = {k: np.ascontiguousarray(np.asarray(v, dtype=np.float32)) for k, v in inputs.items()}
    x = inp["inputs"]
    eps = inp["eps"]
    W_in, b_in = inp["W_in"], inp["b_in"]
    W_h, b_h = inp["W_h"], inp["b_h"]
    W_out, b_out = inp["W_out"], inp["b_out"]

    def cb(a):
        return np.ascontiguousarray(a.astype(bf16))

    c = np.ascontiguousarray

    # wx2: [64, D, 2, 128] with correction rows at partition bands 0 and 32
    wx2 = np.zeros((64, D, 2, 128), np.float32)
    ext = W_in[:, HID:, :]  # [D, 7, 256]
    for s in range(2):
        for j in range(D - 1):
            for m in range(2):
                wx2[32 * s + j, :, m, :] = ext[:, j, 128 * m : 128 * (m + 1)]

    bo_band = np.zeros((128, D), np.float32)
    for j in range(4):
        for ch in range(2):
            bo_band[32 * j + ch, :] = b_out[:, ch]

    wa_np = np.zeros((128, 3360), np.float32)
    wa_np[:IN_DIM, 0:HID] = inp["sW0"]
    wa_np[:64, 256:2304] = wx2.reshape(64, -1)
    wa_np[:, 2304:2816] = inp["sW1"].reshape(2, 128, HID).transpose(1, 0, 2).reshape(128, -1)
    wa_np[:, 2816:3328] = inp["sW2"].reshape(2, 128, HID).transpose(1, 0, 2).reshape(128, -1)
    wa_np[:, 3328:3360] = W_out.reshape(D, 2, 128, 2).transpose(2, 0, 1, 3).reshape(128, -1)
    wbig_np = np.concatenate([
        W_in[:, :HID, :].reshape(D, 2, 128, HID).transpose(2, 0, 1, 3).reshape(128, -1),
        W_h.reshape(D, 2, 128, HID).transpose(2, 0, 1, 3).reshape(128, -1),
    ], axis=1)
    bb_np = np.concatenate([
        inp["sb0"].reshape(2, 128).T, inp["sb1"].reshape(2, 128).T,
        inp["sb2"].reshape(2, 128).T,
        b_in.reshape(D, 2, 128).transpose(2, 0, 1).reshape(128, -1),
        b_h.reshape(D, 2, 128).transpose(2, 0, 1).reshape(128, -1),
        bo_band,
    ], axis=1)

    shared = {
        "wa": cb(wa_np),
        "wbig": cb(wbig_np),
        "bb": c(bb_np),
    }

    in_maps = []
    for core in range(NCORES):
        sl = slice(core * BC, (core + 1) * BC)
        m = dict(shared)
        m["xT"] = cb(x[sl].T)
        m["epsT"] = cb(eps[sl].T)
        in_maps.append(m)

    nc = _get_nc()
    kw = {}
    if TRACE:
        import shutil

        shutil.rmtree("/tmp/ktrace", ignore_errors=True)
        os.makedirs("/tmp/ktrace", exist_ok=True)
        kw = dict(trace=True, trace_cores=[0], tmpdir="/tmp/ktrace")
    res = run_bass_kernel_spmd(nc, in_maps, list(range(NCORES)), **kw)
    if TRACE:
        print(f"HW exec time: {res.exec_time_ns} ns")

    out_mean = np.concatenate([res.results[i]["omT"].T for i in range(NCORES)], axis=0)
    out_sample = np.concatenate([res.results[i]["osT"].T for i in range(NCORES)], axis=0)
    out_logp = np.concatenate([res.results[i]["olT"].T for i in range(NCORES)], axis=0)
    return out_mean, out_sample, out_logp


# revision 22
# speedup vs baseline: 1.4155x; 1.0206x over previous
"""Trainium2 Bass kernel for the autoregressive policy head (nn_ADM_6511170421537).

Structure (per core, pure data parallelism over 8 cores):
  trunk:  h = relu(x@sW0+b) -> relu(@sW1+b) -> relu(@sW2+b)          [B,256]
  steps i=0..7 (sequential in i, batch-parallel):
      x1 = relu(h@W_in[i][:256] + means[:i]@W_in[i][256:256+i] + b_in[i])
      x2 = relu(x1@W_h[i] + b_h[i])
      (mean_i, ls_i) = relu(x2@W_out[i] + b_out[i])
  epilogue (batched over the 8 steps, fp32):
      log_std = min(ls, 2);  std = exp(log_std)
      sample  = mean + std*eps
      logp    = -0.5*eps^2 - log_std - 0.5*log(2pi)   (== reference algebra)

Layout: feature-major on chip ([features->partitions, batch->free]); the host
transposes inputs/eps/outputs so every DMA moves contiguous lines.  Matmuls
run in bf16 (PSUM accumulates fp32), epilogue math in fp32.

Perf structure: batch tiles are processed in GROUPS of 4 (two PAIRS).  All
matmuls sharing a stationary operand are emitted back-to-back (weight-load
hides in the streaming of the previous matmul), a pair shares one 2-bank
PSUM tile so each PSUM->SBUF evacuation covers 2 tiles in one op (FD=1024),
the tiny M=2 head matmuls of the 4 tiles in a group run CONCURRENTLY in
disjoint PE column groups, and the small K=i "autoregressive correction"
matmuls of a pair run concurrently in disjoint PE row groups (the means are
DMA-scattered into per-slot 32-partition bands).
"""

import os

os.environ.setdefault("MYCRO_LOCAL_CACHE", "1")

import numpy as np
from contextlib import ExitStack

import concourse.bass as bass
import concourse.bacc as bacc
import concourse.mybir as mybir
import concourse.tile as tile
from concourse.bass_utils import run_bass_kernel_spmd

# ---- problem constants (hardcoded; kernel.py must be self-contained) ----
B = 65536
IN_DIM = 64
HID = 256
D = 8
NCORES = 8
BC = B // NCORES          # 8192 rows per core
BT = 512                  # batch tile (one fp32 PSUM bank of free dim)
NT = BC // BT             # 16 tiles per core
GRP = 4                   # tiles per group (head col-tiling width)
LOG_2PI = float(np.log(2.0 * np.pi))

F32 = mybir.dt.float32
BF16 = mybir.dt.bfloat16
RELU = mybir.ActivationFunctionType.Relu
EXP = mybir.ActivationFunctionType.Exp
ADD = mybir.AluOpType.add
MAX = mybir.AluOpType.max
MIN = mybir.AluOpType.min
MULT = mybir.AluOpType.mult

TRACE = False           # test.py flips this to get the NTFF profile
_NC_CACHE = {}


def _build_bass():
    nc = bacc.Bacc()

    xT = nc.declare_dram_parameter("xT", [IN_DIM, BC], BF16, isOutput=False)
    epsT = nc.declare_dram_parameter("epsT", [D, BC], BF16, isOutput=False)
    # wa: trunk + small weights [w0pad | wx2pad | w1 | w2 | wo]; wb: [wi | wh]
    wa = nc.declare_dram_parameter("wa", [128, 3360], BF16, isOutput=False)
    wbig = nc.declare_dram_parameter("wbig", [128, 8192], BF16, isOutput=False)
    bb = nc.declare_dram_parameter("bb", [128, 46], F32, isOutput=False)
    omT = nc.declare_dram_parameter("omT", [D, BC], F32, isOutput=True)
    osT = nc.declare_dram_parameter("osT", [D, BC], F32, isOutput=True)
    olT = nc.declare_dram_parameter("olT", [D, BC], F32, isOutput=True)

    with tile.TileContext(nc) as tc, ExitStack() as ctx:
        wp = ctx.enter_context(tc.tile_pool(name="w", bufs=1))
        hpool = ctx.enter_context(tc.tile_pool(name="h", bufs=NT // 2 + 1))
        mlpool = ctx.enter_context(tc.tile_pool(name="ml", bufs=NT + 2))
        xpool = ctx.enter_context(tc.tile_pool(name="xin", bufs=4))
        xtpool = ctx.enter_context(tc.tile_pool(name="xtp", bufs=NT))
        tpool = ctx.enter_context(tc.tile_pool(name="tr", bufs=2))
        opool = ctx.enter_context(tc.tile_pool(name="out", bufs=1))
        pspair = ctx.enter_context(tc.tile_pool(name="pspair", bufs=4, space="PSUM"))

        # ---- batched loads: inputs + trunk weights first (unblock the
        # ---- first matmuls), the big step-weight blob last
        xts_s = wp.tile([IN_DIM, NT, BT], BF16)
        xv = xT[:].rearrange("p (t n) -> p t n", t=NT)
        nc.sync.dma_start(xts_s[:, 0:GRP, :], xv[:, 0:GRP, :])
        xts = [xts_s[:, t, :] for t in range(NT)]
        wa_s = wp.tile([128, 3360], BF16)
        nc.sync.dma_start(wa_s[:], wa[:])
        bb_s = wp.tile([128, 46], F32)
        nc.sync.dma_start(bb_s[:], bb[:])
        nc.sync.dma_start(xts_s[:, GRP:NT, :], xv[:, GRP:NT, :])
        eps_s = wp.tile([D, NT, BT], BF16)
        nc.sync.dma_start(eps_s[:], epsT[:].rearrange("p (t n) -> p t n", t=NT))
        wbig_s = wp.tile([128, 8192], BF16)
        nc.sync.dma_start(wbig_s[:], wbig[:])

        w0_s = wa_s[0:IN_DIM, 0:HID]
        wx2_s = wa_s[0:64, 256:2304].rearrange("j (i m c) -> j i m c", i=D, m=2)
        w1_s = wa_s[:, 2304:2816].rearrange("p (k m) -> p k m", k=2)
        w2_s = wa_s[:, 2816:3328].rearrange("p (k m) -> p k m", k=2)
        wo_s = wa_s[:, 3328:3360].rearrange("p (i k c) -> p i k c", i=D, k=2)
        wi_s = wbig_s[:, 0:4096].rearrange("p (i k m) -> p i k m", i=D, k=2)
        wh_s = wbig_s[:, 4096:8192].rearrange("p (i k m) -> p i k m", i=D, k=2)
        b0_s = bb_s[:, 0:2]
        b1_s = bb_s[:, 2:4]
        b2_s = bb_s[:, 4:6]
        bi_s = bb_s[:, 6:22].rearrange("p (i m) -> p i m", i=D)
        bh_s = bb_s[:, 22:38].rearrange("p (i m) -> p i m", i=D)
        bo_s = bb_s[:, 38:46]

        def evac_act(dst, src, bias):
            nc.scalar.activation(dst, src, RELU, bias=bias)

        def evac_dve(dst, src, bias):
            nc.vector.tensor_scalar(dst, src, bias, 0.0, ADD, MAX)

        # A "pair tile" holds two batch tiles: SBUF [128, m(2), slot(2), BT];
        # PSUM pair tiles are [128, slot(2), BT] (2 banks).

        def layer_pairs(weight_col, bias_col, rhs_of, dst_of, evacs, n_k, corr_i=0,
                        mlps=None):
            """One dense layer over a group of 2 pairs (4 tiles)."""
            for m in range(2):
                pss = [
                    pspair.tile([128, 2, BT], F32, tag="pspair", name=f"ps{m}{p}")
                    for p in range(2)
                ]
                for k in range(n_k):
                    wv = weight_col(k, m)
                    for p in range(2):
                        for s in range(2):
                            nc.tensor.matmul(
                                pss[p][:, s, :], wv, rhs_of(2 * p + s, k),
                                start=(k == 0), stop=(k == n_k - 1 and corr_i == 0),
                            )
                if corr_i > 0:
                    i = corr_i
                    for p in range(2):
                        for s in range(2):
                            # slot s reads its means band at partitions 32s;
                            # the two slots run in disjoint PE row groups.
                            nc.tensor.matmul(
                                pss[p][:, s, :],
                                wx2_s[32 * s : 32 * s + i, i, m, :],
                                mlps[p][32 * s : 32 * s + i, :],
                                start=False, stop=True,
                                tile_position=(32 * s, 0),
                            )
                for p in range(2):
                    evacs[p](dst_of(p, m), pss[p][:], bias_col(m))

        def trunk_group(g, hps):
            """Trunk for tiles [4g..4g+4); writes h into hps[p] pair tiles."""
            t0 = GRP * g
            hp = [tpool.tile([128, 2, 2, BT], BF16, tag=f"hp{p}", name=f"hp{g}_{p}", bufs=1) for p in range(2)]
            layer_pairs(
                lambda k, m: w0_s[:, bass.ts(m, 128)],
                lambda m: b0_s[:, m : m + 1],
                lambda j, k: xts[t0 + j],
                lambda p, m: hp[p][:, m, :, :],
                [evac_act, evac_dve],
                n_k=1,
            )
            hq = [tpool.tile([128, 2, 2, BT], BF16, tag=f"hq{p}", name=f"hq{g}_{p}", bufs=1) for p in range(2)]
            layer_pairs(
                lambda k, m: w1_s[:, k, bass.ts(m, 128)],
                lambda m: b1_s[:, m : m + 1],
                lambda j, k: hp[j // 2][:, k, j % 2, :],
                lambda p, m: hq[p][:, m, :, :],
                [evac_dve, evac_act],
                n_k=2,
            )
            layer_pairs(
                lambda k, m: w2_s[:, k, bass.ts(m, 128)],
                lambda m: b2_s[:, m : m + 1],
                lambda j, k: hq[j // 2][:, k, j % 2, :],
                lambda p, m: hps[p][:, m, :, :],
                [evac_act, evac_dve],
                n_k=2,
            )

        def step_mlps(i, g, hps, mlps):
            """Step i MLP part (L_in + L_h) for the 4 tiles of group g."""
            x1 = [tpool.tile([128, 2, 2, BT], BF16, tag=f"x1{p}", name=f"x1{g}_{p}") for p in range(2)]
            layer_pairs(
                lambda k, m: wi_s[:, i, k, bass.ts(m, 128)],
                lambda m: bi_s[:, i, m : m + 1],
                lambda j, k: hps[j // 2][:, k, j % 2, :],
                lambda p, m: x1[p][:, m, :, :],
                [evac_act, evac_dve],
                n_k=2,
                corr_i=i,
                mlps=mlps,
            )
            x2 = [tpool.tile([128, 2, 2, BT], BF16, tag=f"x2{p}", name=f"x2{g}_{p}") for p in range(2)]
            layer_pairs(
                lambda k, m: wh_s[:, i, k, bass.ts(m, 128)],
                lambda m: bh_s[:, i, m : m + 1],
                lambda j, k: x1[j // 2][:, k, j % 2, :],
                lambda p, m: x2[p][:, m, :, :],
                [evac_dve, evac_act],
                n_k=2,
            )
            return x2

        def step_head(i, g, x2, mlps):
            # head: the 4 tiles' M=2 matmuls run concurrently in distinct
            # PE column groups (tile_position), landing at psum partitions 32j.
            pst = pspair.tile([128, 2, BT], F32, tag="pspair", name=f"psh{g}")
            pso = pst[:, 0, :]
            for k in range(2):
                for j in range(GRP):
                    nc.tensor.matmul(
                        pso[32 * j : 32 * j + 2, :],
                        wo_s[:, i, k, :],
                        x2[j // 2][:, k, j % 2, :],
                        start=(k == 0), stop=(k == 1),
                        tile_position=(0, 32 * j),
                    )
            sm = xpool.tile([128, BT], BF16, tag="sm", name=f"sm{g}_{i}")
            evac_act(sm[0:98, :], pso[0:98, :], bo_s[0:98, i : i + 1])
            for j in range(GRP):
                p, s = j // 2, j % 2
                nc.sync.dma_start(
                    mlps[p][32 * s + i : 32 * s + i + 1, :], sm[32 * j : 32 * j + 1, :]
                )
                nc.gpsimd.dma_start(
                    mlps[p][64 + 32 * s + i : 64 + 32 * s + i + 1, :],
                    sm[32 * j + 1 : 32 * j + 2, :],
                )

        def epilogue_pair(g, p, mlp):
            """Epilogue for pair p of group g (two tiles, FD=1024 ops)."""
            t0 = GRP * g + 2 * p
            et = eps_s[:, t0 : t0 + 2, :]
            mean_f = opool.tile([D, 2, BT], F32, tag="mean_f")
            nc.vector.tensor_copy(mean_f[:, 0, :], mlp[0:D, :])
            nc.vector.tensor_copy(mean_f[:, 1, :], mlp[32 : 32 + D, :])
            ls = opool.tile([D, 2, BT], BF16, tag="ls")
            nc.vector.tensor_single_scalar(ls[:, 0, :], mlp[64 : 64 + D, :], 2.0, MIN)
            nc.vector.tensor_single_scalar(ls[:, 1, :], mlp[96 : 96 + D, :], 2.0, MIN)
            st = opool.tile([D, 2, BT], BF16, tag="st")
            nc.scalar.activation(st[:], ls[:], EXP)
            nc.sync.dma_start(omT[:, bass.ts(t0 // 2, 2 * BT)], mean_f[:])
            # elementwise in-place: DVE writes trail reads through the pipe.
            # mean_f carries mean -> sample -> logp between the output DMAs.
            nc.vector.tensor_mul(st[:], st[:], et)               # std*eps (bf16 2x)
            smp = opool.tile([D, 2, BT], F32, tag="smp")
            nc.vector.tensor_add(smp[:], st[:], mean_f[:])       # sample -> f32
            nc.sync.dma_start(osT[:, bass.ts(t0 // 2, 2 * BT)], smp[:])
            sq = opool.tile([D, 2, BT], BF16, tag="sq")
            nc.vector.tensor_mul(sq[:], et, et)                  # eps^2 (bf16 2x)
            nc.vector.tensor_scalar(sq[:], sq[:], -0.5, -0.5 * LOG_2PI, MULT, ADD)
            lp = opool.tile([D, 2, BT], F32, tag="lp")
            nc.vector.tensor_sub(lp[:], sq[:], ls[:])            # logp -> f32
            nc.sync.dma_start(olT[:, bass.ts(t0 // 2, 2 * BT)], lp[:])

        NG = NT // GRP  # 4 groups
        WAVEG = 2       # groups per wave
        state = {}
        for g in range(NG):
            state[g] = dict(
                h=[hpool.tile([128, 2, 2, BT], BF16, tag="h", name=f"h{g}_{p}") for p in range(2)],
                mlp=[mlpool.tile([128, BT], BF16, tag="mlp", name=f"mlp{g}_{p}") for p in range(2)],
            )

        for wv in range(NG // WAVEG):
            groups = list(range(wv * WAVEG, (wv + 1) * WAVEG))
            if wv == 0:
                for g in groups:
                    trunk_group(g, state[g]["h"])
            for i in range(D):
                x2s = {}
                for g in groups:
                    st_ = state[g]
                    x2s[g] = step_mlps(i, g, st_["h"], st_["mlp"])
                for g in groups:
                    st_ = state[g]
                    step_head(i, g, x2s[g], st_["mlp"])
                # emit next wave's trunk early so the PE has work across
                # the wave boundary
                if i == D - 2 and wv + 1 < NG // WAVEG:
                    for g2 in range((wv + 1) * WAVEG, (wv + 2) * WAVEG):
                        trunk_group(g2, state[g2]["h"])
                if i == D - 1:
                    for g in groups:
                        for p in range(2):
                            epilogue_pair(g, p, state[g]["mlp"][p])

    nc.compile()
    return nc


def _get_nc():
    if "nc" not in _NC_CACHE:
        _NC_CACHE["nc"] = _build_bass()
    return _NC_CACHE["nc"]


def kernel(**inputs):
    import ml_dtypes

    bf16 = ml_dtypes.bfloat16
    inp = {k: np.ascontiguousarray(np.asarray(v, dtype=np.float32)) for k, v in inputs.items()}
    x = inp["inputs"]
    eps = inp["eps"]
    W_in, b_in = inp["W_in"], inp["b_in"]
    W_h, b_h = inp["W_h"], inp["b_h"]
    W_out, b_out = inp["W_out"], inp["b_out"]

    def cb(a):
        return np.ascontiguousarray(a.astype(bf16))

    c = np.ascontiguousarray

    # wx2: [64, D, 2, 128] with correction rows at partition bands 0 and 32
    wx2 = np.zeros((64, D, 2, 128), np.float32)
    ext = W_in[:, HID:, :]  # [D, 7, 256]
    for s in range(2):
        for j in range(D - 1):
            for m in range(2):
                wx2[32 * s + j, :, m, :] = ext[:, j, 128 * m : 128 * (m + 1)]

    bo_band = np.zeros((128, D), np.float32)
    for j in range(4):
        for ch in range(2):
            bo_band[32 * j + ch, :] = b_out[:, ch]

    wa_np = np.zeros((128, 3360), np.float32)
    wa_np[:IN_DIM, 0:HID] = inp["sW0"]
    wa_np[:64, 256:2304] = wx2.reshape(64, -1)
    wa_np[:, 2304:2816] = inp["sW1"].reshape(2, 128, HID).transpose(1, 0, 2).reshape(128, -1)
    wa_np[:, 2816:3328] = inp["sW2"].reshape(2, 128, HID).transpose(1, 0, 2).reshape(128, -1)
    wa_np[:, 3328:3360] = W_out.reshape(D, 2, 128, 2).transpose(2, 0, 1, 3).reshape(128, -1)
    wbig_np = np.concatenate([
        W_in[:, :HID, :].reshape(D, 2, 128, HID).transpose(2, 0, 1, 3).reshape(128, -1),
        W_h.reshape(D, 2, 128, HID).transpose(2, 0, 1, 3).reshape(128, -1),
    ], axis=1)
    bb_np = np.concatenate([
        inp["sb0"].reshape(2, 128).T, inp["sb1"].reshape(2, 128).T,
        inp["sb2"].reshape(2, 128).T,
        b_in.reshape(D, 2, 128).transpose(2, 0, 1).reshape(128, -1),
        b_h.reshape(D, 2, 128).transpose(2, 0, 1).reshape(128, -1),
        bo_band,
    ], axis=1)

    shared = {
        "wa": cb(wa_np),
        "wbig": cb(wbig_np),
        "bb": c(bb_np),
    }

    in_maps = []
    for core in range(NCORES):
        sl = slice(core * BC, (core + 1) * BC)
        m = dict(shared)
        m["xT"] = cb(x[sl].T)
        m["epsT"] = cb(eps[sl].T)
        in_maps.append(m)

    nc = _get_nc()
    kw = {}
    if TRACE:
        import shutil

        shutil.rmtree("/tmp/ktrace", ignore_errors=True)
        os.makedirs("/tmp/ktrace", exist_ok=True)
        kw = dict(trace=True, trace_cores=[0], tmpdir="/tmp/ktrace")
    res = run_bass_kernel_spmd(nc, in_maps, list(range(NCORES)), **kw)
    if TRACE:
        print(f"HW exec time: {res.exec_time_ns} ns")

    out_mean = np.concatenate([res.results[i]["omT"].T for i in range(NCORES)], axis=0)
    out_sample = np.concatenate([res.results[i]["osT"].T for i in range(NCORES)], axis=0)
    out_logp = np.concatenate([res.results[i]["olT"].T for i in range(NCORES)], axis=0)
    return out_mean, out_sample, out_logp
